# revision 69
# baseline (speedup 1.0000x reference)
"""Trainium2 Bass kernel for nn_Attention (dense transformer block).

Reference computation (per batch b):
  pe   = BN(dwconv3x3(x))                     # depthwise positional encoding
  qk   = SiLU(BN(conv1x1(x, qkv_w)))          # -> q (256ch), k (256ch)
  v    = x + pe
  attn = softmax(q^T k / sqrt(32)) per head (8 heads, d=32)
  out  = SiLU(BN(conv1x1(attn_out, proj_w)))

Sharding: 8 cores = 4 batches x 2 spatial halves (800 query positions each).
Each core computes all heads for its query half; no collectives needed.

Pipeline design:
  - per slot: one QK pair (2 heads), one exp, and AV fillers; exp alternates
    between ScalarE (exact table exp) and DVE (Schraudolph fast-exp:
    i16 = trunc(a*s + b), bitcast bf16) to use both engines.
  - QK uses a K=64 zero-padded k/q layout (head pair at rows 0:32 / 64:96)
    so each QK pair lights up all four PE row-quadrants; this keeps the HAM
    clock gate mostly at 2.4GHz at zero extra stream cycles.
  - QK scores stream through a 3-deep ring of [128,2,512] psum buffers.
  - AV for group g-1 is front-loaded (2/slot) into group g's early slots so
    the denominator normalize finishes inside group g and frees the AV psum;
    the final group also runs its own AV in its late slots to keep the drain
    tail short.  dwconv + qkv convs fill group 0 (which has no AV).
  - v^T is produced by writing v to DRAM in a head-interleaved 34-row-stride
    layout (with constant-one rows) and DMA-transposing per key tile, so the
    AV matmul also accumulates the softmax denominator.
  - normalize: DVE reciprocal of the denominator rows, K=64 ones-matmul
    broadcast across 32 partitions (bf16, borrowed ring tile; K padded so
    the HAM clock gate sees full row activity at group boundaries), ScalarE
    psum->sbuf copy, then DVE multiplies into the bf16 proj input.
  - all conv SiLUs are pinned before the first exp (one act-table load each
    way); small shift vectors ride in one packed [128,8] DMA.
  - BN shift of pe is folded into the proj bias (softmax weights sum to 1);
    the center dwconv tap is folded into the v = x + pe elementwise op.
"""

import math
import os
import sys

sys.path.insert(0, "/opt/trn_rl_repo")

import numpy as np
import ml_dtypes

BF16 = ml_dtypes.bfloat16
EPS = 1e-5

C = 256          # channels
N = 1600         # spatial positions (40x40)
NPAD = 1664      # keys padded to 13*128
PW = 42          # padded width/height for dwconv
PADN = PW * PW   # 1764
NH = 8           # heads
D = 32           # head dim
I = 800          # query positions per core
SCALE = float(D) ** -0.5
JT = 13          # number of 128-row key tiles (12*128 + 64)
IC = [(0, 512), (512, 288)]
GROUPS = [(0, 0), (0, 1), (1, 0), (1, 1)]  # (icx, head-group)
VSTRIDE = 34     # per-head row stride in the v^T DRAM bounce (32 v + 1 one + 1 pad)
VROWS = VSTRIDE * NH  # 272

# Schraudolph fast-exp constants (bf16 bit pattern via int16):
# i16 = trunc(s * EXPA + EXPB); bitcast(i16) ~= exp(SCALE * s).
EXPA = SCALE * 128.0 / math.log(2.0)
EXPB = 127.0 * 128.0 - 5.0 + 0.5   # magic offset C=5.0; +0.5 compensates trunc

LAST_EXEC_NS = None
_NC_CACHE = None


def _dve_half(icx, g, h):
    """Which exp halves go to the DVE (Schraudolph) vs ScalarE (exact).

    The previous group's normalize (recip + 4 tensor-muls, ~3.6us of DVE)
    lands in a known slot window of each group; route those slots' exps to
    ScalarE so the norm doesn't stall the exp stream.
    """
    gi = 2 * icx + g
    norm_win = {1: range(20, 26), 2: range(18, 24), 3: range(13, 19)}.get(gi, ())
    if h in norm_win:
        return False
    if icx == 0:
        return h % 2 == 1 if gi == 1 else h % 12 in (1, 3, 5, 7, 9)
    return h % 2 == 1


def _build_nc():
    import concourse.bass as bass  # noqa: F401
    import concourse.mybir as mybir
    import concourse.tile as tile
    from concourse import bacc
    from contextlib import ExitStack

    dt = mybir.dt
    AF = mybir.ActivationFunctionType
    ALU = mybir.AluOpType

    nc = bacc.Bacc(
        "TRN2", target_bir_lowering=False, debug=False, num_devices=8
    )

    x_d = nc.declare_dram_parameter("x", [C, N], dt.bfloat16, isOutput=False)
    xq_d = nc.declare_dram_parameter("xq", [C, I], dt.bfloat16, isOutput=False)
    xpad_d = nc.declare_dram_parameter("xpad", [C, PADN], dt.bfloat16, isOutput=False)
    wqkv_d = nc.declare_dram_parameter("wqkvT", [C, 2 * C], dt.bfloat16, isOutput=False)
    wproj_d = nc.declare_dram_parameter("wprojs", [4, 128, C], dt.bfloat16, isOutput=False)
    wpe_d = nc.declare_dram_parameter("wpe8", [128, 16, 128], dt.bfloat16, isOutput=False)
    # packed [128, 8] f32: cols 0-3 qkv shift, 4-5 proj shift, 6-7 (1+w4)
    shpack_d = nc.declare_dram_parameter("shpack", [128, 8], dt.float32, isOutput=False)
    out_d = nc.declare_dram_parameter("out", [C, I], dt.float32, isOutput=True)

    with ExitStack() as ctx:
        tc = ctx.enter_context(tile.TileContext(nc))
        consts = ctx.enter_context(tc.tile_pool(name="consts", bufs=1))
        work = ctx.enter_context(tc.tile_pool(name="work", bufs=2))
        expool = ctx.enter_context(tc.tile_pool(name="expool", bufs=52))
        dram_pool = ctx.enter_context(tc.tile_pool(name="drams", bufs=1, space="DRAM"))
        pp_ring = ctx.enter_context(tc.tile_pool(name="pp_ring", bufs=3, space="PSUM"))
        util_ctx = ExitStack()
        pp_util = util_ctx.enter_context(
            tc.tile_pool(name="pp_util", bufs=2, space="PSUM")
        )
        av_ctx = ExitStack()
        proj_ctx = ExitStack()
        pp_av = None

        # ---------------- input + weight DMAs (sync HW queue, in need-order) ----------------
        wq = []
        for ct in range(2):
            t = consts.tile([128, 2 * C], dt.bfloat16, tag=f"wq{ct}", name=f"wq{ct}")
            nc.sync.dma_start(t[:], wqkv_d.ap()[128 * ct : 128 * (ct + 1), :])
            wq.append(t)
        xb = []
        for ct in range(2):
            t = consts.tile([128, N], dt.bfloat16, tag=f"xb{ct}", name=f"xb{ct}")
            xb.append(t)
        # two pieces per ct so the first conv chunks start on piece 0
        for poff, plen in ((0, 1024), (1024, 576)):
            for ct in range(2):
                nc.sync.dma_start(
                    xb[ct][:, poff : poff + plen],
                    x_d.ap()[128 * ct : 128 * (ct + 1), poff : poff + plen],
                )
        xqb = []
        for ct in range(2):
            t = consts.tile([128, I], dt.bfloat16, tag=f"xqb{ct}", name=f"xqb{ct}")
            nc.sync.dma_start(t[:], xq_d.ap()[128 * ct : 128 * (ct + 1), :])
            xqb.append(t)
        xpad = []
        for ct in range(2):
            t = consts.tile([128, PADN], dt.bfloat16, tag=f"xpad{ct}", name=f"xpad{ct}")
            nc.gpsimd.dma_start(t[:], xpad_d.ap()[128 * ct : 128 * (ct + 1), :])
            xpad.append(t)
        shpack = consts.tile([128, 8], dt.float32, tag="shpack", name="shpack")
        nc.sync.dma_start(shpack[:], shpack_d.ap())
        shq = [shpack[:, ot : ot + 1] for ot in range(4)]  # 0,1: q; 2,3: k
        shpj = [shpack[:, 4 + ot : 5 + ot] for ot in range(2)]
        w4p1 = [shpack[:, 6 + ct : 7 + ct] for ct in range(2)]
        wpe = consts.tile([128, 16, 128], dt.bfloat16, tag="wpe", name="wpe")
        nc.sync.dma_start(wpe[:], wpe_d.ap())
        wpr = []
        for p in range(4):
            t = consts.tile([128, C], dt.bfloat16, tag=f"wpr{p}", name=f"wpr{p}")
            nc.sync.dma_start(t[:], wproj_d.ap()[p, :, :])
            wpr.append(t)

        # v tiles and the ones rows for the v^T bounce
        vb = []
        for ct in range(2):
            t = consts.tile([128, NPAD], dt.bfloat16, tag=f"vb{ct}", name=f"vb{ct}")
            nc.gpsimd.memset(t[:, N:NPAD], 0.0)
            vb.append(t)
        ones16 = consts.tile([16, NPAD], dt.bfloat16, tag="ones16", name="ones16")
        nc.gpsimd.memset(ones16[:], 1.0)
        # broadcast weights: zeros except all-ones rows at partitions 32/96,
        # so the K=64 norm-broadcast matmuls cover full row-quadrant pairs
        # (keeps HAM activity up at group boundaries)
        bcones = consts.tile([128, 32], dt.bfloat16, tag="bcones", name="bcones")
        nc.vector.memset(bcones[:], 0.0)
        nc.vector.memset(bcones[32:33, :], 1.0)
        nc.vector.memset(bcones[96:97, :], 1.0)
        vdram = dram_pool.tile([VROWS, NPAD], dt.bfloat16, tag="vdram", name="vdram")
        # rows 34h+32 (ones) and 34h+33 (pad) of every head
        nc.sync.dma_start(
            vdram[:].rearrange("(h e) w -> h e w", e=VSTRIDE)[:, 32:34, :], ones16[:]
        )

        # output accumulators (bf16, memset once; junk-safe rows stay zero)
        oT_all = {}
        for icx in range(2):
            ts = []
            for p in range(4):
                t = work.tile(
                    [128, 512], dt.bfloat16, tag=f"oT{icx}_{p}", name=f"oT{icx}_{p}",
                    bufs=1,
                )
                nc.gpsimd.memset(t[:], 0.0)
                ts.append(t)
            oT_all[icx] = ts

        # per-key-tile v^T tiles
        vbT = []
        for jt in range(JT):
            t = consts.tile([128, VROWS], dt.bfloat16, tag=f"vbT{jt}", name=f"vbT{jt}")
            vbT.append(t)

        kb = []
        qb = []
        for ot in range(2):
            kb.append(consts.tile([128, N], dt.bfloat16, tag=f"kb{ot}", name=f"kb{ot}"))
            qb.append(consts.tile([128, I], dt.bfloat16, tag=f"qb{ot}", name=f"qb{ot}"))
        # K=64 zero-padded head layout: per pr-half, head 2pr at rows 0:32 and
        # head 2pr+1 at rows 64:96 (rows 32:64 / 96:128 zero).  The QK pair
        # then lights up all four PE row-quadrants, which keeps the HAM clock
        # gate at full rate; the padded rows add zero extra stream cycles.
        kbp = []
        qbp = []
        for g2 in range(2):
            kt = consts.tile([128, 2, N], dt.bfloat16, tag=f"kbp{g2}", name=f"kbp{g2}")
            qt = consts.tile([128, 2, I], dt.bfloat16, tag=f"qbp{g2}", name=f"qbp{g2}")
            for zlo in (32, 96):
                # DVE memsets: the gpsimd queue is backlogged at startup and
                # slot 0's QK gates on these zero rows
                nc.vector.memset(kt[zlo : zlo + 32, :, :], 0.0)
                nc.vector.memset(qt[zlo : zlo + 32, :, :], 0.0)
            kbp.append(kt)
            qbp.append(qt)

        K_CHUNKS = [(0, 512), (512, 512), (1024, 512), (1536, 64)]
        conv_silus = []
        _conv_ps_spare = []

        def conv_ps():
            # conv accumulators ride in ring-tile halves: 6 in flight (vs 2
            # on pp_util), so the conv chain isn't latency-bound on psum reuse
            if _conv_ps_spare:
                return _conv_ps_spare.pop()
            t = pp_ring.tile([128, 2, 512], dt.float32, tag="qk", name="qk")
            _conv_ps_spare.append(t[:, 1, :])
            return t[:, 0, :]

        def emit_conv_k(ot, chunks=(0, 1, 2, 3)):
            for ci in chunks:
                off, cs = K_CHUNKS[ci]
                ps = conv_ps()
                for ct in range(2):
                    nc.tensor.matmul(
                        ps[:, :cs],
                        wq[ct][:, C + 128 * ot : C + 128 * (ot + 1)],
                        xb[ct][:, off : off + cs],
                        start=(ct == 0),
                        stop=(ct == 1),
                    )
                conv_silus.append(nc.scalar.activation(
                    kb[ot][:, off : off + cs], ps[:, :cs], AF.Silu,
                    bias=shq[2 + ot], scale=1.0,
                ))

        def emit_conv_q(ot, icx):
            ic_off, ic = IC[icx]
            ps = conv_ps()
            for ct in range(2):
                nc.tensor.matmul(
                    ps[:, :ic],
                    wq[ct][:, 128 * ot : 128 * (ot + 1)],
                    xqb[ct][:, ic_off : ic_off + ic],
                    start=(ct == 0),
                    stop=(ct == 1),
                )
            conv_silus.append(nc.scalar.activation(
                qb[ot][:, ic_off : ic_off + ic], ps[:, :ic], AF.Silu,
                bias=shq[ot], scale=1.0,
            ))

        # PE warmup on the first weight tile (trips HAM to 2.4GHz during DMAs)
        for _ in range(4):
            ps = pp_util.tile([128, 512], dt.float32, tag="util", name="util")
            nc.tensor.matmul(ps[:, :], wq[0][:, 0:128], wq[0][:, :], start=True, stop=True)

        # all qkv convs up front (SiLUs grouped before the exp table load).
        # The k/q bounce into the K=64 zero-padded layout is pipelined with
        # the conv chunks for group 0 (sync queue) so slot 0 starts early;
        # group 1's bounce rides the idle gpsimd queue (needed ~26 slots in).
        def emit_kb_bounce(g2, poff, plen, q):
            for pr2 in range(2):
                for k2 in range(2):
                    hl = 2 * pr2 + k2
                    q.dma_start(
                        kbp[g2][64 * k2 : 64 * k2 + 32, pr2, poff : poff + plen],
                        kb[g2][32 * hl : 32 * hl + 32, poff : poff + plen],
                    )

        def emit_qb_bounce(g2, poff, plen, q):
            for pr2 in range(2):
                for k2 in range(2):
                    hl = 2 * pr2 + k2
                    q.dma_start(
                        qbp[g2][64 * k2 : 64 * k2 + 32, pr2, poff : poff + plen],
                        qb[g2][32 * hl : 32 * hl + 32, poff : poff + plen],
                    )

        emit_conv_k(0, (0,))
        emit_kb_bounce(0, 0, 512, nc.sync)
        emit_conv_q(0, 0)
        emit_qb_bounce(0, 0, 512, nc.sync)
        emit_conv_k(0, (1,))
        emit_kb_bounce(0, 512, 512, nc.sync)
        emit_conv_k(0, (2, 3))
        emit_kb_bounce(0, 1024, 576, nc.sync)
        emit_conv_q(0, 1)
        emit_qb_bounce(0, 512, 288, nc.sync)
        emit_conv_k(1)
        emit_conv_q(1, 0)
        emit_conv_q(1, 1)
        emit_kb_bounce(1, 0, N, nc.gpsimd)
        emit_qb_bounce(1, 0, I, nc.gpsimd)

        # ---------------- filler generators ----------------
        # dwconv (8 non-center taps) + v = x*(1+w4) + pe8, bounced to vdram,
        # then per-key-tile DMA transposes into vbT.
        ROW_CHUNKS = [(0, 12), (12, 12), (24, 12), (36, 4)]
        TAPS8 = [0, 1, 2, 3, 5, 6, 7, 8]

        dw_last = [None]  # last dwconv MM of the most recent items

        def dwconv_gen():
            done_a = [False, False]  # per-ct: cols 0..960 DMA'd
            done_b = [False, False]
            emitted_t1 = False
            emitted_t2 = False
            order = [(0, 0), (0, 1), (1, 0), (1, 1), (0, 2), (0, 3), (1, 2), (1, 3)]
            for ct, chi in order:
                r0, nr = ROW_CHUNKS[chi]
                ps = pp_util.tile([128, 512], dt.float32, tag="util", name="util")
                for ti, tap in enumerate(TAPS8):
                    dh, dw = tap // 3, tap % 3
                    src = xpad[ct][:].rearrange("p (h w) -> p h w", h=PW)[
                        :, r0 + dh : r0 + dh + nr, dw : dw + 40
                    ]
                    dw_last[0] = nc.tensor.matmul(
                        ps[:, : nr * 40],
                        wpe[:, 8 * ct + ti, :],
                        src,
                        start=(ti == 0),
                        stop=(ti == 7),
                    )
                    yield
                nc.vector.scalar_tensor_tensor(
                    vb[ct][:, 40 * r0 : 40 * (r0 + nr)],
                    xb[ct][:, 40 * r0 : 40 * (r0 + nr)],
                    w4p1[ct],
                    ps[:, : nr * 40],
                    op0=ALU.mult,
                    op1=ALU.add,
                )
                if chi == 1:
                    # cols 0..960 of this ct complete -> bounce to vdram
                    nc.sync.dma_start(
                        vdram[:]
                        .rearrange("(h e) w -> h e w", e=VSTRIDE)[
                            4 * ct : 4 * ct + 4, 0:32, 0:960
                        ],
                        vb[ct][:, 0:960],
                    )
                    done_a[ct] = True
                if chi == 3:
                    nc.sync.dma_start(
                        vdram[:]
                        .rearrange("(h e) w -> h e w", e=VSTRIDE)[
                            4 * ct : 4 * ct + 4, 0:32, 960:NPAD
                        ],
                        vb[ct][:, 960:NPAD],
                    )
                    done_b[ct] = True
                if all(done_a) and not emitted_t1:
                    for jt in range(7):
                        nc.sync.dma_start_transpose(
                            vbT[jt][:], vdram[:, 128 * jt : 128 * (jt + 1)]
                        )
                    emitted_t1 = True
                if all(done_b) and not emitted_t2:
                    for jt in range(7, JT):
                        nc.sync.dma_start_transpose(
                            vbT[jt][:], vdram[:, 128 * jt : 128 * (jt + 1)]
                        )
                    emitted_t2 = True
                yield

        # ---------------- attention pipeline ----------------
        proj_ctx = ExitStack()
        pp_proj = None

        def emit_av_pair(p_icx, p_g, p_exs, avts, jt, pr2):
            ic_off, ic = IC[p_icx]
            js = 128 if jt < 12 else 64
            exb = p_exs[2 * jt + pr2][:].bitcast(dt.bfloat16)
            mms = []
            for k2 in range(2):
                hg = 4 * p_g + 2 * pr2 + k2
                mms.append(nc.tensor.matmul(
                    avts[pr2][64 * k2 : 64 * k2 + 33, 0:ic],
                    vbT[jt][0:js, VSTRIDE * hg : VSTRIDE * hg + 33],
                    exb[0:js, k2, 0:ic],
                    start=(jt == 0),
                    stop=(jt == 12),
                    tile_position=(0, 64 * k2),
                    skip_group_check=True,
                ))
            return mms

        def emit_norm_phase1(p_icx, p_g, avts):
            # denominator reciprocal (DVE) -> K=64 ones-matmul broadcast across
            # 32 partitions (PE, borrowed ring tile) -> psum->sbuf copy (ScalarE)
            ic_off, ic = IC[p_icx]
            rb = pp_ring.tile([128, 2, 512], dt.float32, tag="qk", name="qk")
            for t in range(2):
                rstk = work.tile([128, 512], dt.float32, tag="rstk", name="rstk")
                nc.vector.reciprocal_approx_fast(rstk[0:128, 0:ic], avts[t][0:128, 0:ic])
                # bf16 copy so the broadcast matmul avoids slow fp32 PE mode
                rstk16 = work.tile([128, 512], dt.bfloat16, tag="rstk16", name="rstk16")
                nc.vector.tensor_copy(rstk16[0:128, 0:ic], rstk[0:128, 0:ic])
                for sub, base in enumerate((0, 64)):
                    nc.tensor.matmul(
                        rb[64 * sub : 64 * sub + 32, t, 0:ic],
                        bcones[base : base + 64, 0:32],
                        rstk16[base : base + 64, 0:ic],
                        start=True,
                        stop=True,
                        tile_position=(base, 64 * sub),
                        skip_group_check=True,
                    )
            bc = work.tile([96, 2, 512], dt.float32, tag="bc", name="bc")
            # DVE copy: the norm-window exps are routed to ScalarE, so the DVE
            # queue runs this sooner and releases the borrowed ring tile fast
            nc.vector.tensor_copy(bc[0:96, :, 0:ic], rb[0:96, :, 0:ic])
            return bc

        def emit_norm_phase2(p_icx, p_g, avts, bc):
            ic_off, ic = IC[p_icx]
            oTs = oT_all[p_icx]
            for t in range(2):
                p = 2 * p_g + t
                for sub in range(2):
                    nc.vector.tensor_mul(
                        oTs[p][64 * sub : 64 * sub + 32, 0:ic],
                        avts[t][64 * sub : 64 * sub + 32, 0:ic],
                        bc[64 * sub : 64 * sub + 32, t, 0:ic],
                    )

        from concourse.tile_rust import add_dep_helper

        def pin_after(ins_list, anchor):
            if anchor is not None:
                for mm in ins_list:
                    add_dep_helper(mm.ins, anchor.ins, sync=False,
                                   reason="slot ordering")

        def emit_proj(icx, pin=True):
            # borrows a ring buffer for the accumulation (runs at a group
            # boundary; ring-3 cushion absorbs the brief QK stall)
            ic_off, ic = IC[icx]
            oTs = oT_all[icx]
            for ot in range(2):
                ps = pp_ring.tile([128, 2, 512], dt.float32, tag="qk", name="qk")[
                    :, 0, :
                ]
                for p in range(4):
                    nc.tensor.matmul(
                        ps[:, 0:ic],
                        wpr[p][:, 128 * ot : 128 * (ot + 1)],
                        oTs[p][:, 0:ic],
                        start=(p == 0),
                        stop=(p == 3),
                    )
                ob = work.tile([128, 512], dt.float32, tag="ob", name="ob")
                silu_ins = nc.scalar.activation(
                    ob[:, 0:ic], ps[:, 0:ic], AF.Silu, bias=shpj[ot], scale=1.0
                )
                if pin and last_exp[0] is not None:
                    # keep proj SiLUs after the final exp so the scheduler never
                    # interleaves them into the exp stream (act-table thrash)
                    add_dep_helper(silu_ins.ins, last_exp[0].ins, sync=False,
                                   reason="proj silu after exp stream")
                nc.sync.dma_start(
                    out_d.ap()[128 * ot : 128 * (ot + 1), ic_off : ic_off + ic],
                    ob[:, 0:ic],
                )

        dw_it = dwconv_gen()

        def pop(it, n):
            if it is None:
                return None
            for _ in range(n):
                try:
                    next(it)
                except StopIteration:
                    return None
            return it

        pending = None  # (icx, g, exs)
        avts = None
        norm_pending = None
        last_exp = [None]
        copy_pin = [None]
        for gi, (icx, g) in enumerate(GROUPS):
            ic_off, ic = IC[icx]
            exs = []
            if gi == 1:
                util_ctx.close()
                pp_av = av_ctx.enter_context(
                    tc.tile_pool(name="pp_av", bufs=1, space="PSUM")
                )
                avts = [
                    pp_av.tile([128, 512], dt.float32, tag=f"av{t}", name=f"av{t}",
                               bufs=1)
                    for t in range(2)
                ]
                # rows 33:64 / 97:128 are never matmul-written; init for the
                # normalize reads
                for t in range(2):
                    nc.vector.memset(avts[t][:], 1.0)

            # AV pair schedule for the pending group: front-loaded (two pairs
            # per early slot) so the last pair + normalize land well before the
            # group ends and the avts psum is free for the next group
            av_sched = {}
            if pending is not None:
                pairs = [(j, pr) for j in range(JT) for pr in range(2)]
                start_h = 6 if gi == 1 else 2
                end_h = 22 if gi == 1 else (15 if gi == 3 else 20)
                slots = list(range(start_h, end_h))
                extra = len(pairs) - len(slots)
                pi = 0
                for si, hh in enumerate(slots):
                    take = 2 if si < extra else 1
                    av_sched[hh] = pairs[pi : pi + take]
                    pi += take

            slot_anchor = {}  # h -> last filler instruction of that slot
            own_q = [(j, pr) for j in range(JT) for pr in range(2)] if gi == 3 else []
            prev_norm_done = False
            for h in range(26):
                jt, pr = divmod(h, 2)
                js = 128 if jt < 12 else 64
                rb = pp_ring.tile([128, 2, 512], dt.float32, tag="qk", name="qk")
                qk_mms = []
                for k2 in range(2):
                    qk_mms.append(nc.tensor.matmul(
                        rb[0:js, k2, 0:ic],
                        kbp[g][64 * k2 : 64 * k2 + 64, pr, 128 * jt : 128 * jt + js],
                        qbp[g][64 * k2 : 64 * k2 + 64, pr, ic_off : ic_off + ic],
                        start=True,
                        stop=True,
                        tile_position=(64 * k2, 0),
                    ))
                # keep the PE stream alternating: this half's QK runs after
                # slot h-2's fillers
                pin_after(qk_mms, slot_anchor.get(h - 2))
                ex = expool.tile([128, 2, 512], dt.bfloat16, tag="ex", name="ex")
                if _dve_half(icx, g, h):
                    nc.vector.tensor_scalar(
                        ex[:].bitcast(dt.int16)[0:js, :, 0:ic],
                        rb[0:js, :, 0:ic],
                        EXPA,
                        EXPB,
                        op0=ALU.mult,
                        op1=ALU.add,
                    )
                else:
                    last_exp[0] = nc.scalar.activation(
                        ex[0:js, :, 0:ic], rb[0:js, :, 0:ic], AF.Exp, scale=SCALE
                    )
                    if conv_silus:
                        # force every conv SiLU before the first exp so the
                        # scheduler never thrashes the activation table set
                        for si in conv_silus:
                            add_dep_helper(last_exp[0].ins, si.ins, sync=False,
                                           reason="silu before exp stream")
                        conv_silus.clear()
                exs.append(ex)

                # PE slack fillers: all of dwconv lands inside group 0 (popped
                # fast so the v^T bounce + transposes finish well before the
                # front-loaded AV of group 1 needs them; the first slots are
                # kept dwconv-free so the exp stream ramps immediately)
                if gi == 0 and dw_it is not None and h >= 3:
                    dw_it = pop(dw_it, 8)
                if norm_pending is not None and (h == 1 or h >= norm_pending[4]):
                    emit_norm_phase2(*norm_pending[:4])
                    norm_pending = None
                    prev_norm_done = True
                for j, pr in av_sched.get(h, ()):
                    av_mms = emit_av_pair(
                        pending[0], pending[1], pending[2], avts, j, pr
                    )
                    pin_after(av_mms, qk_mms[1])
                    slot_anchor[h] = av_mms[-1]
                    if (j, pr) == (JT - 1, 1):
                        # AV block done: kick off the denominator reciprocal +
                        # broadcast right away; phase2 lands ~3 slots later,
                        # freeing avts before the next group needs it
                        bc = emit_norm_phase1(pending[0], pending[1], avts)
                        norm_pending = (pending[0], pending[1], avts, bc, h + 3)
                if gi == 3 and prev_norm_done and own_q:
                    # previous group normalized: start the final group's own AV
                    # in its remaining slots instead of a long serial drain
                    took = 0
                    while own_q and took < 3 and own_q[0][0] * 2 + own_q[0][1] <= h - 2:
                        j2, pr2 = own_q.pop(0)
                        av_mms = emit_av_pair(icx, g, exs, avts, j2, pr2)
                        pin_after(av_mms, qk_mms[1])
                        slot_anchor[h] = av_mms[-1]
                        took += 1

            pending = (icx, g, exs)

        # drain fillers (shouldn't be any left, but be safe)
        while dw_it is not None:
            dw_it = pop(dw_it, 8)
        # final group's AV + normalize + both projs
        if norm_pending is not None:
            emit_norm_phase2(*norm_pending[:4])
            norm_pending = None
        p_icx, p_g, p_exs = pending
        # prefetch the SiLU act-table set while the norm chain runs: a 1-elem
        # dummy silu right after the last exp hides the ~1.3us table load
        dummy = work.tile([1, 1], dt.float32, tag="dummy", name="dummy")
        dummy_silu = nc.scalar.activation(
            dummy[0:1, 0:1], shpack[0:1, 0:1], AF.Silu, scale=1.0
        )
        if last_exp[0] is not None:
            add_dep_helper(dummy_silu.ins, last_exp[0].ins, sync=False,
                           reason="table prefetch after exp stream")
        for j, pr in own_q:
            emit_av_pair(p_icx, p_g, p_exs, avts, j, pr)
        bc = emit_norm_phase1(p_icx, p_g, avts)
        emit_proj(0)
        emit_norm_phase2(p_icx, p_g, avts, bc)
        emit_proj(1)
        av_ctx.close()

    nc.compile()
    return nc


def _get_nc():
    global _NC_CACHE
    if _NC_CACHE is None:
        _NC_CACHE = _build_nc()
    return _NC_CACHE


def _prep_weights(inputs):
    f32 = np.float32
    qkv_w = np.asarray(inputs["qkv_w"], f32)
    qinv = np.asarray(inputs["qkv_gamma"], f32) / np.sqrt(
        np.asarray(inputs["qkv_var"], f32) + EPS
    )
    wqkvT = np.ascontiguousarray((qkv_w * qinv[:, None]).T.astype(BF16))
    shqkv = (
        np.asarray(inputs["qkv_beta"], f32) - np.asarray(inputs["qkv_mean"], f32) * qinv
    ).astype(f32)[:, None]

    pe_w = np.asarray(inputs["pe_w"], f32)  # [256, 1, 3, 3]
    peinv = np.asarray(inputs["pe_gamma"], f32) / np.sqrt(
        np.asarray(inputs["pe_var"], f32) + EPS
    )
    wpe_f = (pe_w[:, 0] * peinv[:, None, None]).reshape(C, 9)
    shpe = (
        np.asarray(inputs["pe_beta"], f32) - np.asarray(inputs["pe_mean"], f32) * peinv
    ).astype(f32)
    taps8 = [0, 1, 2, 3, 5, 6, 7, 8]
    wpe8 = np.zeros((16, 128, 128), BF16)
    for ct in range(2):
        for ti, tap in enumerate(taps8):
            np.fill_diagonal(
                wpe8[8 * ct + ti], wpe_f[128 * ct : 128 * (ct + 1), tap].astype(BF16)
            )
    # partition-major so the device DMA is contiguous (no strided rearrange)
    wpe8 = np.ascontiguousarray(wpe8.transpose(1, 0, 2))
    w4p1 = (1.0 + wpe_f[:, 4]).astype(f32)[:, None]

    proj_w = np.asarray(inputs["proj_w"], f32)
    pinv = np.asarray(inputs["proj_gamma"], f32) / np.sqrt(
        np.asarray(inputs["proj_var"], f32) + EPS
    )
    wfold = proj_w * pinv[:, None]          # [out, in]
    wprojT = wfold.T.astype(f32)            # [in, out]
    wprojs = np.zeros((4, 128, C), BF16)
    for p in range(4):
        wprojs[p, 0:32] = wprojT[64 * p : 64 * p + 32].astype(BF16)
        wprojs[p, 64:96] = wprojT[64 * p + 32 : 64 * p + 64].astype(BF16)
    # fold v's BN shift through proj: softmax weights sum to one, so the
    # constant shpe offset on v becomes wfold @ shpe added to the proj bias.
    shproj = (
        np.asarray(inputs["proj_beta"], f32)
        - np.asarray(inputs["proj_mean"], f32) * pinv
        + wfold @ shpe
    ).astype(f32)[:, None]

    # packed [128, 8]: cols 0-3 shqkv chunks, 4-5 shproj chunks, 6-7 w4p1
    shpack = np.zeros((128, 8), f32)
    for ot in range(4):
        shpack[:, ot] = shqkv[128 * ot : 128 * (ot + 1), 0]
    for ot in range(2):
        shpack[:, 4 + ot] = shproj[128 * ot : 128 * (ot + 1), 0]
    for ct in range(2):
        shpack[:, 6 + ct] = w4p1[128 * ct : 128 * (ct + 1), 0]

    return dict(wqkvT=wqkvT, wprojs=wprojs, wpe8=wpe8, shpack=shpack)


def build_in_maps(inputs):
    w = _prep_weights(inputs)
    x = np.asarray(inputs["x"], np.float32)  # [4, 256, 40, 40]
    in_maps = []
    for core in range(8):
        b, hf = divmod(core, 2)
        xr = np.ascontiguousarray(x[b].reshape(C, N))
        xb16 = xr.astype(BF16)
        xp = np.zeros((C, PW, PW), BF16)
        xp[:, 1:41, 1:41] = xb16.reshape(C, 40, 40)
        m = {
            "x": np.ascontiguousarray(xb16),
            "xq": np.ascontiguousarray(xb16[:, I * hf : I * (hf + 1)]),
            "xpad": np.ascontiguousarray(xp.reshape(C, PADN)),
        }
        m.update(w)
        in_maps.append(m)
    return in_maps


def assemble(results):
    out = np.empty((4, C, 40, 40), np.float32)
    for core in range(8):
        b, hf = divmod(core, 2)
        o = np.asarray(results[core]["out"], np.float32)
        out[b].reshape(C, N)[:, I * hf : I * (hf + 1)] = o
    return out


def _install_ntff_hook():
    """Provide antenv.axon_hooks (missing in this image) so trace=True works."""
    import types

    try:
        import antenv.axon_hooks  # noqa: F401
        return
    except ImportError:
        pass
    import antenv

    mod = types.ModuleType("antenv.axon_hooks")
    state = {"hook": None}
    mod.set_axon_ntff_profile_hook = lambda h: state.__setitem__("hook", h)
    mod.get_axon_ntff_profile_hook = lambda: state["hook"]
    sys.modules["antenv.axon_hooks"] = mod
    antenv.axon_hooks = mod

    so_path = "/opt/axon/libaxon_pjrt.so"
    if os.path.exists(so_path):
        boot_dir = "/root/.axon_site/trn_agent_boot"
        if boot_dir not in sys.path and os.path.isdir(boot_dir):
            sys.path.append(boot_dir)
        try:
            from trn_boot import _ntff_profile_via_ctypes

            mod.set_axon_ntff_profile_hook(_ntff_profile_via_ctypes(so_path))
        except Exception as e:  # pragma: no cover
            print(f"ntff hook install failed: {e}", file=sys.stderr)


def kernel(**inputs):
    global LAST_EXEC_NS
    _install_ntff_hook()
    from concourse.bass_utils import run_bass_kernel_spmd

    nc = _get_nc()
    in_maps = build_in_maps(inputs)
    trace = bool(int(os.environ.get("KERNEL_TRACE", "0")))
    res = run_bass_kernel_spmd(nc, in_maps, core_ids=list(range(8)), trace=trace)
    LAST_EXEC_NS = res.exec_time_ns
    return assemble(res.results)



# revision 72
# speedup vs baseline: 1.0042x; 1.0042x over previous
"""Trainium2 Bass kernel for nn_Attention (dense transformer block).

Reference computation (per batch b):
  pe   = BN(dwconv3x3(x))                     # depthwise positional encoding
  qk   = SiLU(BN(conv1x1(x, qkv_w)))          # -> q (256ch), k (256ch)
  v    = x + pe
  attn = softmax(q^T k / sqrt(32)) per head (8 heads, d=32)
  out  = SiLU(BN(conv1x1(attn_out, proj_w)))

Sharding: 8 cores = 4 batches x 2 spatial halves (800 query positions each).
Each core computes all heads for its query half; no collectives needed.

Pipeline design:
  - per slot: one QK pair (2 heads), one exp, and AV fillers; exp alternates
    between ScalarE (exact table exp) and DVE (Schraudolph fast-exp:
    i16 = trunc(a*s + b), bitcast bf16) to use both engines.
  - QK uses a K=64 zero-padded k/q layout (head pair at rows 0:32 / 64:96)
    so each QK pair lights up all four PE row-quadrants; this keeps the HAM
    clock gate mostly at 2.4GHz at zero extra stream cycles.
  - QK scores stream through a 3-deep ring of [128,2,512] psum buffers.
  - AV for group g-1 is front-loaded (2/slot) into group g's early slots so
    the denominator normalize finishes inside group g and frees the AV psum;
    the final group also runs its own AV in its late slots to keep the drain
    tail short.  dwconv + qkv convs fill group 0 (which has no AV).
  - v^T is produced by writing v to DRAM in a head-interleaved 34-row-stride
    layout (with constant-one rows) and DMA-transposing per key tile, so the
    AV matmul also accumulates the softmax denominator.
  - normalize: DVE reciprocal of the denominator rows, K=64 ones-matmul
    broadcast across 32 partitions (bf16, borrowed ring tile; K padded so
    the HAM clock gate sees full row activity at group boundaries), ScalarE
    psum->sbuf copy, then DVE multiplies into the bf16 proj input.
  - all conv SiLUs are pinned before the first exp (one act-table load each
    way); small shift vectors ride in one packed [128,8] DMA.
  - BN shift of pe is folded into the proj bias (softmax weights sum to 1);
    the center dwconv tap is folded into the v = x + pe elementwise op.
"""

import math
import os
import sys

sys.path.insert(0, "/opt/trn_rl_repo")

import numpy as np
import ml_dtypes

BF16 = ml_dtypes.bfloat16
EPS = 1e-5

C = 256          # channels
N = 1600         # spatial positions (40x40)
NPAD = 1664      # keys padded to 13*128
PW = 42          # padded width/height for dwconv
PADN = PW * PW   # 1764
NH = 8           # heads
D = 32           # head dim
I = 800          # query positions per core
SCALE = float(D) ** -0.5
JT = 13          # number of 128-row key tiles (12*128 + 64)
IC = [(0, 512), (512, 288)]
GROUPS = [(0, 0), (0, 1), (1, 0), (1, 1)]  # (icx, head-group)
VSTRIDE = 34     # per-head row stride in the v^T DRAM bounce (32 v + 1 one + 1 pad)
VROWS = VSTRIDE * NH  # 272

# Schraudolph fast-exp constants (bf16 bit pattern via int16):
# i16 = trunc(s * EXPA + EXPB); bitcast(i16) ~= exp(SCALE * s).
EXPA = SCALE * 128.0 / math.log(2.0)
EXPB = 127.0 * 128.0 - 5.0 + 0.5   # magic offset C=5.0; +0.5 compensates trunc

LAST_EXEC_NS = None
_NC_CACHE = None


def _dve_half(icx, g, h):
    """Which exp halves go to the DVE (Schraudolph) vs ScalarE (exact).

    The previous group's normalize (recip + 4 tensor-muls, ~3.6us of DVE)
    lands in a known slot window of each group; route those slots' exps to
    ScalarE so the norm doesn't stall the exp stream.
    """
    gi = 2 * icx + g
    norm_win = {1: range(20, 26), 2: range(18, 24), 3: range(13, 19)}.get(gi, ())
    if h in norm_win:
        return False
    if icx == 0:
        return h % 2 == 1 if gi == 1 else h % 12 in (1, 3, 5, 7, 9)
    return h % 2 == 1


def _build_nc():
    import concourse.bass as bass  # noqa: F401
    import concourse.mybir as mybir
    import concourse.tile as tile
    from concourse import bacc
    from contextlib import ExitStack

    dt = mybir.dt
    AF = mybir.ActivationFunctionType
    ALU = mybir.AluOpType

    nc = bacc.Bacc(
        "TRN2", target_bir_lowering=False, debug=False, num_devices=8
    )

    x_d = nc.declare_dram_parameter("x", [C, N], dt.bfloat16, isOutput=False)
    xq_d = nc.declare_dram_parameter("xq", [C, I], dt.bfloat16, isOutput=False)
    xpad_d = nc.declare_dram_parameter("xpad", [C, PADN], dt.bfloat16, isOutput=False)
    wqkv_d = nc.declare_dram_parameter("wqkvT", [C, 2 * C], dt.bfloat16, isOutput=False)
    wproj_d = nc.declare_dram_parameter("wprojs", [4, 128, C], dt.bfloat16, isOutput=False)
    wpe_d = nc.declare_dram_parameter("wpe8", [128, 16, 128], dt.bfloat16, isOutput=False)
    # packed [128, 8] f32: cols 0-3 qkv shift, 4-5 proj shift, 6-7 (1+w4)
    shpack_d = nc.declare_dram_parameter("shpack", [128, 8], dt.float32, isOutput=False)
    out_d = nc.declare_dram_parameter("out", [C, I], dt.float32, isOutput=True)

    with ExitStack() as ctx:
        tc = ctx.enter_context(tile.TileContext(nc))
        consts = ctx.enter_context(tc.tile_pool(name="consts", bufs=1))
        work = ctx.enter_context(tc.tile_pool(name="work", bufs=2))
        expool = ctx.enter_context(tc.tile_pool(name="expool", bufs=52))
        dram_pool = ctx.enter_context(tc.tile_pool(name="drams", bufs=1, space="DRAM"))
        pp_ring = ctx.enter_context(tc.tile_pool(name="pp_ring", bufs=3, space="PSUM"))
        util_ctx = ExitStack()
        pp_util = util_ctx.enter_context(
            tc.tile_pool(name="pp_util", bufs=2, space="PSUM")
        )
        av_ctx = ExitStack()
        proj_ctx = ExitStack()
        pp_av = None

        # ---------------- input + weight DMAs (sync HW queue, in need-order) ----------------
        wq = []
        for ct in range(2):
            t = consts.tile([128, 2 * C], dt.bfloat16, tag=f"wq{ct}", name=f"wq{ct}")
            nc.sync.dma_start(t[:], wqkv_d.ap()[128 * ct : 128 * (ct + 1), :])
            wq.append(t)
        xb = []
        for ct in range(2):
            t = consts.tile([128, N], dt.bfloat16, tag=f"xb{ct}", name=f"xb{ct}")
            xb.append(t)
        # two pieces per ct so the first conv chunks start on piece 0
        for poff, plen in ((0, 1024), (1024, 576)):
            for ct in range(2):
                nc.sync.dma_start(
                    xb[ct][:, poff : poff + plen],
                    x_d.ap()[128 * ct : 128 * (ct + 1), poff : poff + plen],
                )
        xqb = []
        for ct in range(2):
            t = consts.tile([128, I], dt.bfloat16, tag=f"xqb{ct}", name=f"xqb{ct}")
            nc.sync.dma_start(t[:], xq_d.ap()[128 * ct : 128 * (ct + 1), :])
            xqb.append(t)
        xpad = []
        for ct in range(2):
            t = consts.tile([128, PADN], dt.bfloat16, tag=f"xpad{ct}", name=f"xpad{ct}")
            nc.gpsimd.dma_start(t[:], xpad_d.ap()[128 * ct : 128 * (ct + 1), :])
            xpad.append(t)
        shpack = consts.tile([128, 8], dt.float32, tag="shpack", name="shpack")
        nc.sync.dma_start(shpack[:], shpack_d.ap())
        shq = [shpack[:, ot : ot + 1] for ot in range(4)]  # 0,1: q; 2,3: k
        shpj = [shpack[:, 4 + ot : 5 + ot] for ot in range(2)]
        w4p1 = [shpack[:, 6 + ct : 7 + ct] for ct in range(2)]
        wpe = consts.tile([128, 16, 128], dt.bfloat16, tag="wpe", name="wpe")
        nc.sync.dma_start(wpe[:], wpe_d.ap())
        wpr = []
        for p in range(4):
            t = consts.tile([128, C], dt.bfloat16, tag=f"wpr{p}", name=f"wpr{p}")
            nc.sync.dma_start(t[:], wproj_d.ap()[p, :, :])
            wpr.append(t)

        # v tiles and the ones rows for the v^T bounce
        vb = []
        for ct in range(2):
            t = consts.tile([128, NPAD], dt.bfloat16, tag=f"vb{ct}", name=f"vb{ct}")
            nc.gpsimd.memset(t[:, N:NPAD], 0.0)
            vb.append(t)
        ones16 = consts.tile([16, NPAD], dt.bfloat16, tag="ones16", name="ones16")
        nc.gpsimd.memset(ones16[:], 1.0)
        # broadcast weights: zeros except all-ones rows at partitions 32/96,
        # so the K=64 norm-broadcast matmuls cover full row-quadrant pairs
        # (keeps HAM activity up at group boundaries)
        bcones = consts.tile([128, 32], dt.bfloat16, tag="bcones", name="bcones")
        nc.vector.memset(bcones[:], 0.0)
        nc.vector.memset(bcones[32:33, :], 1.0)
        nc.vector.memset(bcones[96:97, :], 1.0)
        vdram = dram_pool.tile([VROWS, NPAD], dt.bfloat16, tag="vdram", name="vdram")
        # rows 34h+32 (ones) and 34h+33 (pad) of every head
        nc.sync.dma_start(
            vdram[:].rearrange("(h e) w -> h e w", e=VSTRIDE)[:, 32:34, :], ones16[:]
        )

        # output accumulators (bf16, memset once; junk-safe rows stay zero)
        oT_all = {}
        for icx in range(2):
            ts = []
            for p in range(4):
                t = work.tile(
                    [128, 512], dt.bfloat16, tag=f"oT{icx}_{p}", name=f"oT{icx}_{p}",
                    bufs=1,
                )
                nc.gpsimd.memset(t[:], 0.0)
                ts.append(t)
            oT_all[icx] = ts

        # per-key-tile v^T tiles
        vbT = []
        for jt in range(JT):
            t = consts.tile([128, VROWS], dt.bfloat16, tag=f"vbT{jt}", name=f"vbT{jt}")
            vbT.append(t)

        kb = []
        qb = []
        for ot in range(2):
            kb.append(consts.tile([128, N], dt.bfloat16, tag=f"kb{ot}", name=f"kb{ot}"))
            qb.append(consts.tile([128, I], dt.bfloat16, tag=f"qb{ot}", name=f"qb{ot}"))
        # K=64 zero-padded head layout: per pr-half, head 2pr at rows 0:32 and
        # head 2pr+1 at rows 64:96 (rows 32:64 / 96:128 zero).  The QK pair
        # then lights up all four PE row-quadrants, which keeps the HAM clock
        # gate at full rate; the padded rows add zero extra stream cycles.
        kbp = []
        qbp = []
        for g2 in range(2):
            kt = consts.tile([128, 2, N], dt.bfloat16, tag=f"kbp{g2}", name=f"kbp{g2}")
            qt = consts.tile([128, 2, I], dt.bfloat16, tag=f"qbp{g2}", name=f"qbp{g2}")
            for zlo in (32, 96):
                # DVE memsets: the gpsimd queue is backlogged at startup and
                # slot 0's QK gates on these zero rows
                nc.vector.memset(kt[zlo : zlo + 32, :, :], 0.0)
                nc.vector.memset(qt[zlo : zlo + 32, :, :], 0.0)
            kbp.append(kt)
            qbp.append(qt)

        K_CHUNKS = [(0, 512), (512, 512), (1024, 512), (1536, 64)]
        conv_silus = []
        _conv_ps_spare = []

        def conv_ps():
            # conv accumulators ride in ring-tile halves: 6 in flight (vs 2
            # on pp_util), so the conv chain isn't latency-bound on psum reuse
            if _conv_ps_spare:
                return _conv_ps_spare.pop()
            t = pp_ring.tile([128, 2, 512], dt.float32, tag="qk", name="qk")
            _conv_ps_spare.append(t[:, 1, :])
            return t[:, 0, :]

        def emit_conv_k(ot, chunks=(0, 1, 2, 3)):
            for ci in chunks:
                off, cs = K_CHUNKS[ci]
                ps = conv_ps()
                for ct in range(2):
                    nc.tensor.matmul(
                        ps[:, :cs],
                        wq[ct][:, C + 128 * ot : C + 128 * (ot + 1)],
                        xb[ct][:, off : off + cs],
                        start=(ct == 0),
                        stop=(ct == 1),
                    )
                conv_silus.append(nc.scalar.activation(
                    kb[ot][:, off : off + cs], ps[:, :cs], AF.Silu,
                    bias=shq[2 + ot], scale=1.0,
                ))

        def emit_conv_q(ot, icx):
            ic_off, ic = IC[icx]
            ps = conv_ps()
            for ct in range(2):
                nc.tensor.matmul(
                    ps[:, :ic],
                    wq[ct][:, 128 * ot : 128 * (ot + 1)],
                    xqb[ct][:, ic_off : ic_off + ic],
                    start=(ct == 0),
                    stop=(ct == 1),
                )
            conv_silus.append(nc.scalar.activation(
                qb[ot][:, ic_off : ic_off + ic], ps[:, :ic], AF.Silu,
                bias=shq[ot], scale=1.0,
            ))

        # PE warmup on the first weight tile (trips HAM to 2.4GHz during DMAs)
        for _ in range(4):
            ps = pp_util.tile([128, 512], dt.float32, tag="util", name="util")
            nc.tensor.matmul(ps[:, :], wq[0][:, 0:128], wq[0][:, :], start=True, stop=True)

        # all qkv convs up front (SiLUs grouped before the exp table load).
        # The k/q bounce into the K=64 zero-padded layout is pipelined with
        # the conv chunks for group 0 (sync queue) so slot 0 starts early;
        # group 1's bounce rides the idle gpsimd queue (needed ~26 slots in).
        def emit_kb_bounce(g2, poff, plen, q):
            for pr2 in range(2):
                for k2 in range(2):
                    hl = 2 * pr2 + k2
                    q.dma_start(
                        kbp[g2][64 * k2 : 64 * k2 + 32, pr2, poff : poff + plen],
                        kb[g2][32 * hl : 32 * hl + 32, poff : poff + plen],
                    )

        def emit_qb_bounce(g2, poff, plen, q):
            for pr2 in range(2):
                for k2 in range(2):
                    hl = 2 * pr2 + k2
                    q.dma_start(
                        qbp[g2][64 * k2 : 64 * k2 + 32, pr2, poff : poff + plen],
                        qb[g2][32 * hl : 32 * hl + 32, poff : poff + plen],
                    )

        emit_conv_k(0, (0,))
        emit_kb_bounce(0, 0, 512, nc.sync)
        emit_conv_q(0, 0)
        emit_qb_bounce(0, 0, 512, nc.sync)
        emit_conv_k(0, (1,))
        emit_kb_bounce(0, 512, 512, nc.sync)
        emit_conv_k(0, (2, 3))
        emit_kb_bounce(0, 1024, 576, nc.sync)
        emit_conv_q(0, 1)
        emit_qb_bounce(0, 512, 288, nc.sync)
        emit_conv_k(1)
        emit_conv_q(1, 0)
        emit_conv_q(1, 1)
        emit_kb_bounce(1, 0, N, nc.gpsimd)
        emit_qb_bounce(1, 0, I, nc.gpsimd)

        # ---------------- filler generators ----------------
        # dwconv (8 non-center taps) + v = x*(1+w4) + pe8, bounced to vdram,
        # then per-key-tile DMA transposes into vbT.
        ROW_CHUNKS = [(0, 12), (12, 12), (24, 12), (36, 4)]
        TAPS8 = [0, 1, 2, 3, 5, 6, 7, 8]

        dw_last = [None]  # last dwconv MM of the most recent items
        dw_mms_buf = []   # dwconv MMs emitted since the last clear (for pins)

        def dwconv_gen():
            done_a = [False, False]  # per-ct: cols 0..960 DMA'd
            done_b = [False, False]
            emitted_t1 = False
            emitted_t2 = False
            order = [(0, 0), (0, 1), (1, 0), (1, 1), (0, 2), (0, 3), (1, 2), (1, 3)]
            for ct, chi in order:
                r0, nr = ROW_CHUNKS[chi]
                ps = pp_util.tile([128, 512], dt.float32, tag="util", name="util")
                for ti, tap in enumerate(TAPS8):
                    dh, dw = tap // 3, tap % 3
                    src = xpad[ct][:].rearrange("p (h w) -> p h w", h=PW)[
                        :, r0 + dh : r0 + dh + nr, dw : dw + 40
                    ]
                    dw_last[0] = nc.tensor.matmul(
                        ps[:, : nr * 40],
                        wpe[:, 8 * ct + ti, :],
                        src,
                        start=(ti == 0),
                        stop=(ti == 7),
                    )
                    dw_mms_buf.append(dw_last[0])
                    yield
                nc.vector.scalar_tensor_tensor(
                    vb[ct][:, 40 * r0 : 40 * (r0 + nr)],
                    xb[ct][:, 40 * r0 : 40 * (r0 + nr)],
                    w4p1[ct],
                    ps[:, : nr * 40],
                    op0=ALU.mult,
                    op1=ALU.add,
                )
                if chi == 1:
                    # cols 0..960 of this ct complete -> bounce to vdram
                    nc.sync.dma_start(
                        vdram[:]
                        .rearrange("(h e) w -> h e w", e=VSTRIDE)[
                            4 * ct : 4 * ct + 4, 0:32, 0:960
                        ],
                        vb[ct][:, 0:960],
                    )
                    done_a[ct] = True
                if chi == 3:
                    nc.sync.dma_start(
                        vdram[:]
                        .rearrange("(h e) w -> h e w", e=VSTRIDE)[
                            4 * ct : 4 * ct + 4, 0:32, 960:NPAD
                        ],
                        vb[ct][:, 960:NPAD],
                    )
                    done_b[ct] = True
                if all(done_a) and not emitted_t1:
                    for jt in range(7):
                        nc.sync.dma_start_transpose(
                            vbT[jt][:], vdram[:, 128 * jt : 128 * (jt + 1)]
                        )
                    emitted_t1 = True
                if all(done_b) and not emitted_t2:
                    for jt in range(7, JT):
                        nc.sync.dma_start_transpose(
                            vbT[jt][:], vdram[:, 128 * jt : 128 * (jt + 1)]
                        )
                    emitted_t2 = True
                yield

        # ---------------- attention pipeline ----------------
        proj_ctx = ExitStack()
        pp_proj = None

        def emit_av_pair(p_icx, p_g, p_exs, avts, jt, pr2):
            ic_off, ic = IC[p_icx]
            js = 128 if jt < 12 else 64
            exb = p_exs[2 * jt + pr2][:].bitcast(dt.bfloat16)
            mms = []
            for k2 in range(2):
                hg = 4 * p_g + 2 * pr2 + k2
                mms.append(nc.tensor.matmul(
                    avts[pr2][64 * k2 : 64 * k2 + 33, 0:ic],
                    vbT[jt][0:js, VSTRIDE * hg : VSTRIDE * hg + 33],
                    exb[0:js, k2, 0:ic],
                    start=(jt == 0),
                    stop=(jt == 12),
                    tile_position=(0, 64 * k2),
                    skip_group_check=True,
                ))
            return mms

        def emit_norm_phase1(p_icx, p_g, avts):
            # denominator reciprocal (DVE) -> K=64 ones-matmul broadcast across
            # 32 partitions (PE, borrowed ring tile) -> psum->sbuf copy (ScalarE)
            ic_off, ic = IC[p_icx]
            rb = pp_ring.tile([128, 2, 512], dt.float32, tag="qk", name="qk")
            for t in range(2):
                rstk = work.tile([128, 512], dt.float32, tag="rstk", name="rstk")
                nc.vector.reciprocal_approx_fast(rstk[0:128, 0:ic], avts[t][0:128, 0:ic])
                # bf16 copy so the broadcast matmul avoids slow fp32 PE mode
                rstk16 = work.tile([128, 512], dt.bfloat16, tag="rstk16", name="rstk16")
                nc.vector.tensor_copy(rstk16[0:128, 0:ic], rstk[0:128, 0:ic])
                for sub, base in enumerate((0, 64)):
                    nc.tensor.matmul(
                        rb[64 * sub : 64 * sub + 32, t, 0:ic],
                        bcones[base : base + 64, 0:32],
                        rstk16[base : base + 64, 0:ic],
                        start=True,
                        stop=True,
                        tile_position=(base, 64 * sub),
                        skip_group_check=True,
                    )
            bc = work.tile([96, 2, 512], dt.float32, tag="bc", name="bc")
            # DVE copy: the norm-window exps are routed to ScalarE, so the DVE
            # queue runs this sooner and releases the borrowed ring tile fast
            nc.vector.tensor_copy(bc[0:96, :, 0:ic], rb[0:96, :, 0:ic])
            return bc

        def emit_norm_phase2(p_icx, p_g, avts, bc):
            ic_off, ic = IC[p_icx]
            oTs = oT_all[p_icx]
            for t in range(2):
                p = 2 * p_g + t
                for sub in range(2):
                    nc.vector.tensor_mul(
                        oTs[p][64 * sub : 64 * sub + 32, 0:ic],
                        avts[t][64 * sub : 64 * sub + 32, 0:ic],
                        bc[64 * sub : 64 * sub + 32, t, 0:ic],
                    )

        from concourse.tile_rust import add_dep_helper

        def pin_after(ins_list, anchor):
            if anchor is not None:
                for mm in ins_list:
                    add_dep_helper(mm.ins, anchor.ins, sync=False,
                                   reason="slot ordering")

        def emit_proj(icx, pin=True):
            # borrows a ring buffer for the accumulation (runs at a group
            # boundary; ring-3 cushion absorbs the brief QK stall)
            ic_off, ic = IC[icx]
            oTs = oT_all[icx]
            for ot in range(2):
                ps = pp_ring.tile([128, 2, 512], dt.float32, tag="qk", name="qk")[
                    :, 0, :
                ]
                for p in range(4):
                    nc.tensor.matmul(
                        ps[:, 0:ic],
                        wpr[p][:, 128 * ot : 128 * (ot + 1)],
                        oTs[p][:, 0:ic],
                        start=(p == 0),
                        stop=(p == 3),
                    )
                ob = work.tile([128, 512], dt.float32, tag="ob", name="ob")
                silu_ins = nc.scalar.activation(
                    ob[:, 0:ic], ps[:, 0:ic], AF.Silu, bias=shpj[ot], scale=1.0
                )
                if pin and last_exp[0] is not None:
                    # keep proj SiLUs after the final exp so the scheduler never
                    # interleaves them into the exp stream (act-table thrash)
                    add_dep_helper(silu_ins.ins, last_exp[0].ins, sync=False,
                                   reason="proj silu after exp stream")
                nc.sync.dma_start(
                    out_d.ap()[128 * ot : 128 * (ot + 1), ic_off : ic_off + ic],
                    ob[:, 0:ic],
                )

        dw_it = dwconv_gen()

        def pop(it, n):
            if it is None:
                return None
            for _ in range(n):
                try:
                    next(it)
                except StopIteration:
                    return None
            return it

        pending = None  # (icx, g, exs)
        avts = None
        norm_pending = None
        last_exp = [None]
        copy_pin = [None]
        for gi, (icx, g) in enumerate(GROUPS):
            ic_off, ic = IC[icx]
            exs = []
            if gi == 1:
                util_ctx.close()
                pp_av = av_ctx.enter_context(
                    tc.tile_pool(name="pp_av", bufs=1, space="PSUM")
                )
                avts = [
                    pp_av.tile([128, 512], dt.float32, tag=f"av{t}", name=f"av{t}",
                               bufs=1)
                    for t in range(2)
                ]
                # rows 33:64 / 97:128 are never matmul-written; init for the
                # normalize reads
                for t in range(2):
                    nc.vector.memset(avts[t][:], 1.0)

            # AV pair schedule for the pending group: front-loaded (two pairs
            # per early slot) so the last pair + normalize land well before the
            # group ends and the avts psum is free for the next group
            av_sched = {}
            if pending is not None:
                pairs = [(j, pr) for j in range(JT) for pr in range(2)]
                start_h = 6 if gi == 1 else 2
                end_h = 22 if gi == 1 else (15 if gi == 3 else 20)
                slots = list(range(start_h, end_h))
                extra = len(pairs) - len(slots)
                pi = 0
                for si, hh in enumerate(slots):
                    take = 2 if si < extra else 1
                    av_sched[hh] = pairs[pi : pi + take]
                    pi += take

            slot_anchor = {}  # h -> last filler instruction of that slot
            own_q = [(j, pr) for j in range(JT) for pr in range(2)] if gi == 3 else []
            prev_norm_done = False
            for h in range(26):
                jt, pr = divmod(h, 2)
                js = 128 if jt < 12 else 64
                rb = pp_ring.tile([128, 2, 512], dt.float32, tag="qk", name="qk")
                qk_mms = []
                for k2 in range(2):
                    qk_mms.append(nc.tensor.matmul(
                        rb[0:js, k2, 0:ic],
                        kbp[g][64 * k2 : 64 * k2 + 64, pr, 128 * jt : 128 * jt + js],
                        qbp[g][64 * k2 : 64 * k2 + 64, pr, ic_off : ic_off + ic],
                        start=True,
                        stop=True,
                        tile_position=(64 * k2, 0),
                    ))
                # keep the PE stream alternating: this half's QK runs after
                # slot h-2's fillers
                pin_after(qk_mms, slot_anchor.get(h - 2))
                ex = expool.tile([128, 2, 512], dt.bfloat16, tag="ex", name="ex")
                if _dve_half(icx, g, h):
                    nc.vector.tensor_scalar(
                        ex[:].bitcast(dt.int16)[0:js, :, 0:ic],
                        rb[0:js, :, 0:ic],
                        EXPA,
                        EXPB,
                        op0=ALU.mult,
                        op1=ALU.add,
                    )
                else:
                    last_exp[0] = nc.scalar.activation(
                        ex[0:js, :, 0:ic], rb[0:js, :, 0:ic], AF.Exp, scale=SCALE
                    )
                    if conv_silus:
                        # force every conv SiLU before the first exp so the
                        # scheduler never thrashes the activation table set
                        for si in conv_silus:
                            add_dep_helper(last_exp[0].ins, si.ins, sync=False,
                                           reason="silu before exp stream")
                        conv_silus.clear()
                exs.append(ex)

                # PE slack fillers: all of dwconv lands inside group 0 (popped
                # fast so the v^T bounce + transposes finish well before the
                # front-loaded AV of group 1 needs them; the first slots are
                # kept dwconv-free so the exp stream ramps immediately)
                if gi == 0 and dw_it is not None and h >= 3:
                    dw_mms_buf.clear()
                    dw_it = pop(dw_it, 8)
                    # this slot's QK pair gets the array first; dwconv fills in
                    # behind it so the exp stream is never starved
                    pin_after(dw_mms_buf, qk_mms[1])
                    dw_mms_buf.clear()
                if norm_pending is not None and (h == 1 or h >= norm_pending[4]):
                    emit_norm_phase2(*norm_pending[:4])
                    norm_pending = None
                    prev_norm_done = True
                for j, pr in av_sched.get(h, ()):
                    av_mms = emit_av_pair(
                        pending[0], pending[1], pending[2], avts, j, pr
                    )
                    pin_after(av_mms, qk_mms[1])
                    slot_anchor[h] = av_mms[-1]
                    if (j, pr) == (JT - 1, 1):
                        # AV block done: kick off the denominator reciprocal +
                        # broadcast right away; phase2 lands ~3 slots later,
                        # freeing avts before the next group needs it
                        bc = emit_norm_phase1(pending[0], pending[1], avts)
                        norm_pending = (pending[0], pending[1], avts, bc, h + 3)
                if gi == 3 and prev_norm_done and own_q:
                    # previous group normalized: start the final group's own AV
                    # in its remaining slots instead of a long serial drain
                    took = 0
                    while own_q and took < 3 and own_q[0][0] * 2 + own_q[0][1] <= h - 2:
                        j2, pr2 = own_q.pop(0)
                        av_mms = emit_av_pair(icx, g, exs, avts, j2, pr2)
                        pin_after(av_mms, qk_mms[1])
                        slot_anchor[h] = av_mms[-1]
                        took += 1

            pending = (icx, g, exs)

        # drain fillers (shouldn't be any left, but be safe)
        while dw_it is not None:
            dw_it = pop(dw_it, 8)
        # final group's AV + normalize + both projs
        if norm_pending is not None:
            emit_norm_phase2(*norm_pending[:4])
            norm_pending = None
        p_icx, p_g, p_exs = pending
        # prefetch the SiLU act-table set while the norm chain runs: a 1-elem
        # dummy silu right after the last exp hides the ~1.3us table load
        dummy = work.tile([1, 1], dt.float32, tag="dummy", name="dummy")
        dummy_silu = nc.scalar.activation(
            dummy[0:1, 0:1], shpack[0:1, 0:1], AF.Silu, scale=1.0
        )
        if last_exp[0] is not None:
            add_dep_helper(dummy_silu.ins, last_exp[0].ins, sync=False,
                           reason="table prefetch after exp stream")
        for j, pr in own_q:
            emit_av_pair(p_icx, p_g, p_exs, avts, j, pr)
        bc = emit_norm_phase1(p_icx, p_g, avts)
        emit_proj(0)
        emit_norm_phase2(p_icx, p_g, avts, bc)
        emit_proj(1)
        av_ctx.close()

    nc.compile()
    return nc


def _get_nc():
    global _NC_CACHE
    if _NC_CACHE is None:
        _NC_CACHE = _build_nc()
    return _NC_CACHE


def _prep_weights(inputs):
    f32 = np.float32
    qkv_w = np.asarray(inputs["qkv_w"], f32)
    qinv = np.asarray(inputs["qkv_gamma"], f32) / np.sqrt(
        np.asarray(inputs["qkv_var"], f32) + EPS
    )
    wqkvT = np.ascontiguousarray((qkv_w * qinv[:, None]).T.astype(BF16))
    shqkv = (
        np.asarray(inputs["qkv_beta"], f32) - np.asarray(inputs["qkv_mean"], f32) * qinv
    ).astype(f32)[:, None]

    pe_w = np.asarray(inputs["pe_w"], f32)  # [256, 1, 3, 3]
    peinv = np.asarray(inputs["pe_gamma"], f32) / np.sqrt(
        np.asarray(inputs["pe_var"], f32) + EPS
    )
    wpe_f = (pe_w[:, 0] * peinv[:, None, None]).reshape(C, 9)
    shpe = (
        np.asarray(inputs["pe_beta"], f32) - np.asarray(inputs["pe_mean"], f32) * peinv
    ).astype(f32)
    taps8 = [0, 1, 2, 3, 5, 6, 7, 8]
    wpe8 = np.zeros((16, 128, 128), BF16)
    for ct in range(2):
        for ti, tap in enumerate(taps8):
            np.fill_diagonal(
                wpe8[8 * ct + ti], wpe_f[128 * ct : 128 * (ct + 1), tap].astype(BF16)
            )
    # partition-major so the device DMA is contiguous (no strided rearrange)
    wpe8 = np.ascontiguousarray(wpe8.transpose(1, 0, 2))
    w4p1 = (1.0 + wpe_f[:, 4]).astype(f32)[:, None]

    proj_w = np.asarray(inputs["proj_w"], f32)
    pinv = np.asarray(inputs["proj_gamma"], f32) / np.sqrt(
        np.asarray(inputs["proj_var"], f32) + EPS
    )
    wfold = proj_w * pinv[:, None]          # [out, in]
    wprojT = wfold.T.astype(f32)            # [in, out]
    wprojs = np.zeros((4, 128, C), BF16)
    for p in range(4):
        wprojs[p, 0:32] = wprojT[64 * p : 64 * p + 32].astype(BF16)
        wprojs[p, 64:96] = wprojT[64 * p + 32 : 64 * p + 64].astype(BF16)
    # fold v's BN shift through proj: softmax weights sum to one, so the
    # constant shpe offset on v becomes wfold @ shpe added to the proj bias.
    shproj = (
        np.asarray(inputs["proj_beta"], f32)
        - np.asarray(inputs["proj_mean"], f32) * pinv
        + wfold @ shpe
    ).astype(f32)[:, None]

    # packed [128, 8]: cols 0-3 shqkv chunks, 4-5 shproj chunks, 6-7 w4p1
    shpack = np.zeros((128, 8), f32)
    for ot in range(4):
        shpack[:, ot] = shqkv[128 * ot : 128 * (ot + 1), 0]
    for ot in range(2):
        shpack[:, 4 + ot] = shproj[128 * ot : 128 * (ot + 1), 0]
    for ct in range(2):
        shpack[:, 6 + ct] = w4p1[128 * ct : 128 * (ct + 1), 0]

    return dict(wqkvT=wqkvT, wprojs=wprojs, wpe8=wpe8, shpack=shpack)


def build_in_maps(inputs):
    w = _prep_weights(inputs)
    x = np.asarray(inputs["x"], np.float32)  # [4, 256, 40, 40]
    in_maps = []
    for core in range(8):
        b, hf = divmod(core, 2)
        xr = np.ascontiguousarray(x[b].reshape(C, N))
        xb16 = xr.astype(BF16)
        xp = np.zeros((C, PW, PW), BF16)
        xp[:, 1:41, 1:41] = xb16.reshape(C, 40, 40)
        m = {
            "x": np.ascontiguousarray(xb16),
            "xq": np.ascontiguousarray(xb16[:, I * hf : I * (hf + 1)]),
            "xpad": np.ascontiguousarray(xp.reshape(C, PADN)),
        }
        m.update(w)
        in_maps.append(m)
    return in_maps


def assemble(results):
    out = np.empty((4, C, 40, 40), np.float32)
    for core in range(8):
        b, hf = divmod(core, 2)
        o = np.asarray(results[core]["out"], np.float32)
        out[b].reshape(C, N)[:, I * hf : I * (hf + 1)] = o
    return out


def _install_ntff_hook():
    """Provide antenv.axon_hooks (missing in this image) so trace=True works."""
    import types

    try:
        import antenv.axon_hooks  # noqa: F401
        return
    except ImportError:
        pass
    import antenv

    mod = types.ModuleType("antenv.axon_hooks")
    state = {"hook": None}
    mod.set_axon_ntff_profile_hook = lambda h: state.__setitem__("hook", h)
    mod.get_axon_ntff_profile_hook = lambda: state["hook"]
    sys.modules["antenv.axon_hooks"] = mod
    antenv.axon_hooks = mod

    so_path = "/opt/axon/libaxon_pjrt.so"
    if os.path.exists(so_path):
        boot_dir = "/root/.axon_site/trn_agent_boot"
        if boot_dir not in sys.path and os.path.isdir(boot_dir):
            sys.path.append(boot_dir)
        try:
            from trn_boot import _ntff_profile_via_ctypes

            mod.set_axon_ntff_profile_hook(_ntff_profile_via_ctypes(so_path))
        except Exception as e:  # pragma: no cover
            print(f"ntff hook install failed: {e}", file=sys.stderr)


def kernel(**inputs):
    global LAST_EXEC_NS
    _install_ntff_hook()
    from concourse.bass_utils import run_bass_kernel_spmd

    nc = _get_nc()
    in_maps = build_in_maps(inputs)
    trace = bool(int(os.environ.get("KERNEL_TRACE", "0")))
    res = run_bass_kernel_spmd(nc, in_maps, core_ids=list(range(8)), trace=trace)
    LAST_EXEC_NS = res.exec_time_ns
    return assemble(res.results)



# revision 73
# speedup vs baseline: 1.0042x; 1.0000x over previous
"""Trainium2 Bass kernel for nn_Attention (dense transformer block).

Reference computation (per batch b):
  pe   = BN(dwconv3x3(x))                     # depthwise positional encoding
  qk   = SiLU(BN(conv1x1(x, qkv_w)))          # -> q (256ch), k (256ch)
  v    = x + pe
  attn = softmax(q^T k / sqrt(32)) per head (8 heads, d=32)
  out  = SiLU(BN(conv1x1(attn_out, proj_w)))

Sharding: 8 cores = 4 batches x 2 spatial halves (800 query positions each).
Each core computes all heads for its query half; no collectives needed.

Pipeline design:
  - per slot: one QK pair (2 heads), one exp, and AV fillers; exp alternates
    between ScalarE (exact table exp) and DVE (Schraudolph fast-exp:
    i16 = trunc(a*s + b), bitcast bf16) to use both engines.
  - QK uses a K=64 zero-padded k/q layout (head pair at rows 0:32 / 64:96)
    so each QK pair lights up all four PE row-quadrants; this keeps the HAM
    clock gate mostly at 2.4GHz at zero extra stream cycles.
  - QK scores stream through a 3-deep ring of [128,2,512] psum buffers.
  - AV for group g-1 is front-loaded (2/slot) into group g's early slots so
    the denominator normalize finishes inside group g and frees the AV psum;
    the final group also runs its own AV in its late slots to keep the drain
    tail short.  dwconv + qkv convs fill group 0 (which has no AV).
  - v^T is produced by writing v to DRAM in a head-interleaved 34-row-stride
    layout (with constant-one rows) and DMA-transposing per key tile, so the
    AV matmul also accumulates the softmax denominator.
  - normalize: DVE reciprocal of the denominator rows, K=64 ones-matmul
    broadcast across 32 partitions (bf16, borrowed ring tile; K padded so
    the HAM clock gate sees full row activity at group boundaries), ScalarE
    psum->sbuf copy, then DVE multiplies into the bf16 proj input.
  - all conv SiLUs are pinned before the first exp (one act-table load each
    way); small shift vectors ride in one packed [128,8] DMA.
  - BN shift of pe is folded into the proj bias (softmax weights sum to 1);
    the center dwconv tap is folded into the v = x + pe elementwise op.
"""

import math
import os
import sys

sys.path.insert(0, "/opt/trn_rl_repo")

import numpy as np
import ml_dtypes

BF16 = ml_dtypes.bfloat16
EPS = 1e-5

C = 256          # channels
N = 1600         # spatial positions (40x40)
NPAD = 1664      # keys padded to 13*128
PW = 42          # padded width/height for dwconv
PADN = PW * PW   # 1764
NH = 8           # heads
D = 32           # head dim
I = 800          # query positions per core
SCALE = float(D) ** -0.5
JT = 13          # number of 128-row key tiles (12*128 + 64)
IC = [(0, 512), (512, 288)]
GROUPS = [(0, 0), (0, 1), (1, 0), (1, 1)]  # (icx, head-group)
VSTRIDE = 34     # per-head row stride in the v^T DRAM bounce (32 v + 1 one + 1 pad)
VROWS = VSTRIDE * NH  # 272

# Schraudolph fast-exp constants (bf16 bit pattern via int16):
# i16 = trunc(s * EXPA + EXPB); bitcast(i16) ~= exp(SCALE * s).
EXPA = SCALE * 128.0 / math.log(2.0)
EXPB = 127.0 * 128.0 - 5.0 + 0.5   # magic offset C=5.0; +0.5 compensates trunc

LAST_EXEC_NS = None
_NC_CACHE = None


def _dve_half(icx, g, h):
    """Which exp halves go to the DVE (Schraudolph) vs ScalarE (exact).

    The previous group's normalize (recip + 4 tensor-muls, ~3.6us of DVE)
    lands in a known slot window of each group; route those slots' exps to
    ScalarE so the norm doesn't stall the exp stream.
    """
    gi = 2 * icx + g
    norm_win = {1: range(20, 26), 2: range(18, 24), 3: range(13, 19)}.get(gi, ())
    if h in norm_win:
        return False
    if icx == 0:
        return h % 2 == 1 if gi == 1 else h % 12 in (1, 3, 5, 7, 9)
    return h % 2 == 1


def _build_nc():
    import concourse.bass as bass  # noqa: F401
    import concourse.mybir as mybir
    import concourse.tile as tile
    from concourse import bacc
    from contextlib import ExitStack

    dt = mybir.dt
    AF = mybir.ActivationFunctionType
    ALU = mybir.AluOpType

    nc = bacc.Bacc(
        "TRN2", target_bir_lowering=False, debug=False, num_devices=8
    )

    x_d = nc.declare_dram_parameter("x", [C, N], dt.bfloat16, isOutput=False)
    xq_d = nc.declare_dram_parameter("xq", [C, I], dt.bfloat16, isOutput=False)
    xpad_d = nc.declare_dram_parameter("xpad", [C, PADN], dt.bfloat16, isOutput=False)
    wqkv_d = nc.declare_dram_parameter("wqkvT", [C, 2 * C], dt.bfloat16, isOutput=False)
    wproj_d = nc.declare_dram_parameter("wprojs", [4, 128, C], dt.bfloat16, isOutput=False)
    wpe_d = nc.declare_dram_parameter("wpe8", [128, 16, 128], dt.bfloat16, isOutput=False)
    # packed [128, 8] f32: cols 0-3 qkv shift, 4-5 proj shift, 6-7 (1+w4)
    shpack_d = nc.declare_dram_parameter("shpack", [128, 8], dt.float32, isOutput=False)
    out_d = nc.declare_dram_parameter("out", [C, I], dt.float32, isOutput=True)

    with ExitStack() as ctx:
        tc = ctx.enter_context(tile.TileContext(nc))
        consts = ctx.enter_context(tc.tile_pool(name="consts", bufs=1))
        work = ctx.enter_context(tc.tile_pool(name="work", bufs=2))
        expool = ctx.enter_context(tc.tile_pool(name="expool", bufs=52))
        dram_pool = ctx.enter_context(tc.tile_pool(name="drams", bufs=1, space="DRAM"))
        pp_ring = ctx.enter_context(tc.tile_pool(name="pp_ring", bufs=3, space="PSUM"))
        util_ctx = ExitStack()
        pp_util = util_ctx.enter_context(
            tc.tile_pool(name="pp_util", bufs=2, space="PSUM")
        )
        av_ctx = ExitStack()
        proj_ctx = ExitStack()
        pp_av = None

        # ---------------- input + weight DMAs (sync HW queue, in need-order) ----------------
        wq = []
        for ct in range(2):
            t = consts.tile([128, 2 * C], dt.bfloat16, tag=f"wq{ct}", name=f"wq{ct}")
            nc.sync.dma_start(t[:], wqkv_d.ap()[128 * ct : 128 * (ct + 1), :])
            wq.append(t)
        xb = []
        for ct in range(2):
            t = consts.tile([128, N], dt.bfloat16, tag=f"xb{ct}", name=f"xb{ct}")
            xb.append(t)
        # two pieces per ct so the first conv chunks start on piece 0
        for poff, plen in ((0, 1024), (1024, 576)):
            for ct in range(2):
                nc.sync.dma_start(
                    xb[ct][:, poff : poff + plen],
                    x_d.ap()[128 * ct : 128 * (ct + 1), poff : poff + plen],
                )
        xqb = []
        for ct in range(2):
            t = consts.tile([128, I], dt.bfloat16, tag=f"xqb{ct}", name=f"xqb{ct}")
            nc.sync.dma_start(t[:], xq_d.ap()[128 * ct : 128 * (ct + 1), :])
            xqb.append(t)
        xpad = []
        for ct in range(2):
            t = consts.tile([128, PADN], dt.bfloat16, tag=f"xpad{ct}", name=f"xpad{ct}")
            nc.gpsimd.dma_start(t[:], xpad_d.ap()[128 * ct : 128 * (ct + 1), :])
            xpad.append(t)
        shpack = consts.tile([128, 8], dt.float32, tag="shpack", name="shpack")
        nc.sync.dma_start(shpack[:], shpack_d.ap())
        shq = [shpack[:, ot : ot + 1] for ot in range(4)]  # 0,1: q; 2,3: k
        shpj = [shpack[:, 4 + ot : 5 + ot] for ot in range(2)]
        w4p1 = [shpack[:, 6 + ct : 7 + ct] for ct in range(2)]
        wpe = consts.tile([128, 16, 128], dt.bfloat16, tag="wpe", name="wpe")
        nc.sync.dma_start(wpe[:], wpe_d.ap())
        wpr = []
        for p in range(4):
            t = consts.tile([128, C], dt.bfloat16, tag=f"wpr{p}", name=f"wpr{p}")
            nc.sync.dma_start(t[:], wproj_d.ap()[p, :, :])
            wpr.append(t)

        # v tiles and the ones rows for the v^T bounce
        vb = []
        for ct in range(2):
            t = consts.tile([128, NPAD], dt.bfloat16, tag=f"vb{ct}", name=f"vb{ct}")
            nc.gpsimd.memset(t[:, N:NPAD], 0.0)
            vb.append(t)
        ones16 = consts.tile([16, NPAD], dt.bfloat16, tag="ones16", name="ones16")
        nc.gpsimd.memset(ones16[:], 1.0)
        # broadcast weights: zeros except all-ones rows at partitions 32/96,
        # so the K=64 norm-broadcast matmuls cover full row-quadrant pairs
        # (keeps HAM activity up at group boundaries)
        bcones = consts.tile([128, 32], dt.bfloat16, tag="bcones", name="bcones")
        nc.vector.memset(bcones[:], 0.0)
        nc.vector.memset(bcones[32:33, :], 1.0)
        nc.vector.memset(bcones[96:97, :], 1.0)
        vdram = dram_pool.tile([VROWS, NPAD], dt.bfloat16, tag="vdram", name="vdram")
        # rows 34h+32 (ones) and 34h+33 (pad) of every head
        nc.sync.dma_start(
            vdram[:].rearrange("(h e) w -> h e w", e=VSTRIDE)[:, 32:34, :], ones16[:]
        )

        # output accumulators (bf16, memset once; junk-safe rows stay zero)
        oT_all = {}
        for icx in range(2):
            ts = []
            for p in range(4):
                t = work.tile(
                    [128, 512], dt.bfloat16, tag=f"oT{icx}_{p}", name=f"oT{icx}_{p}",
                    bufs=1,
                )
                nc.gpsimd.memset(t[:], 0.0)
                ts.append(t)
            oT_all[icx] = ts

        # per-key-tile v^T tiles
        vbT = []
        for jt in range(JT):
            t = consts.tile([128, VROWS], dt.bfloat16, tag=f"vbT{jt}", name=f"vbT{jt}")
            vbT.append(t)

        kb = []
        qb = []
        for ot in range(2):
            kb.append(consts.tile([128, N], dt.bfloat16, tag=f"kb{ot}", name=f"kb{ot}"))
            qb.append(consts.tile([128, I], dt.bfloat16, tag=f"qb{ot}", name=f"qb{ot}"))
        # K=64 zero-padded head layout: per pr-half, head 2pr at rows 0:32 and
        # head 2pr+1 at rows 64:96 (rows 32:64 / 96:128 zero).  The QK pair
        # then lights up all four PE row-quadrants, which keeps the HAM clock
        # gate at full rate; the padded rows add zero extra stream cycles.
        kbp = []
        qbp = []
        for g2 in range(2):
            kt = consts.tile([128, 2, N], dt.bfloat16, tag=f"kbp{g2}", name=f"kbp{g2}")
            qt = consts.tile([128, 2, I], dt.bfloat16, tag=f"qbp{g2}", name=f"qbp{g2}")
            for zlo in (32, 96):
                # DVE memsets: the gpsimd queue is backlogged at startup and
                # slot 0's QK gates on these zero rows
                nc.vector.memset(kt[zlo : zlo + 32, :, :], 0.0)
                nc.vector.memset(qt[zlo : zlo + 32, :, :], 0.0)
            kbp.append(kt)
            qbp.append(qt)

        K_CHUNKS = [(0, 512), (512, 512), (1024, 512), (1536, 64)]
        conv_silus = []
        _conv_ps_spare = []

        def conv_ps():
            # conv accumulators ride in ring-tile halves: 6 in flight (vs 2
            # on pp_util), so the conv chain isn't latency-bound on psum reuse
            if _conv_ps_spare:
                return _conv_ps_spare.pop()
            t = pp_ring.tile([128, 2, 512], dt.float32, tag="qk", name="qk")
            _conv_ps_spare.append(t[:, 1, :])
            return t[:, 0, :]

        def emit_conv_k(ot, chunks=(0, 1, 2, 3)):
            for ci in chunks:
                off, cs = K_CHUNKS[ci]
                ps = conv_ps()
                for ct in range(2):
                    nc.tensor.matmul(
                        ps[:, :cs],
                        wq[ct][:, C + 128 * ot : C + 128 * (ot + 1)],
                        xb[ct][:, off : off + cs],
                        start=(ct == 0),
                        stop=(ct == 1),
                    )
                conv_silus.append(nc.scalar.activation(
                    kb[ot][:, off : off + cs], ps[:, :cs], AF.Silu,
                    bias=shq[2 + ot], scale=1.0,
                ))

        def emit_conv_q(ot, icx):
            ic_off, ic = IC[icx]
            ps = conv_ps()
            for ct in range(2):
                nc.tensor.matmul(
                    ps[:, :ic],
                    wq[ct][:, 128 * ot : 128 * (ot + 1)],
                    xqb[ct][:, ic_off : ic_off + ic],
                    start=(ct == 0),
                    stop=(ct == 1),
                )
            conv_silus.append(nc.scalar.activation(
                qb[ot][:, ic_off : ic_off + ic], ps[:, :ic], AF.Silu,
                bias=shq[ot], scale=1.0,
            ))

        # PE warmup on the first weight tile (trips HAM to 2.4GHz during DMAs)
        for _ in range(4):
            ps = pp_util.tile([128, 512], dt.float32, tag="util", name="util")
            nc.tensor.matmul(ps[:, :], wq[0][:, 0:128], wq[0][:, :], start=True, stop=True)

        # all qkv convs up front (SiLUs grouped before the exp table load).
        # The k/q bounce into the K=64 zero-padded layout is pipelined with
        # the conv chunks for group 0 (sync queue) so slot 0 starts early;
        # group 1's bounce rides the idle gpsimd queue (needed ~26 slots in).
        def emit_kb_bounce(g2, poff, plen, q):
            for pr2 in range(2):
                for k2 in range(2):
                    hl = 2 * pr2 + k2
                    q.dma_start(
                        kbp[g2][64 * k2 : 64 * k2 + 32, pr2, poff : poff + plen],
                        kb[g2][32 * hl : 32 * hl + 32, poff : poff + plen],
                    )

        def emit_qb_bounce(g2, poff, plen, q):
            for pr2 in range(2):
                for k2 in range(2):
                    hl = 2 * pr2 + k2
                    q.dma_start(
                        qbp[g2][64 * k2 : 64 * k2 + 32, pr2, poff : poff + plen],
                        qb[g2][32 * hl : 32 * hl + 32, poff : poff + plen],
                    )

        emit_conv_k(0, (0,))
        emit_kb_bounce(0, 0, 512, nc.sync)
        emit_conv_q(0, 0)
        emit_qb_bounce(0, 0, 512, nc.sync)
        emit_conv_k(0, (1,))
        emit_kb_bounce(0, 512, 512, nc.sync)
        emit_conv_k(0, (2, 3))
        emit_kb_bounce(0, 1024, 576, nc.sync)
        emit_conv_q(0, 1)
        emit_qb_bounce(0, 512, 288, nc.sync)
        emit_conv_k(1)
        emit_conv_q(1, 0)
        emit_conv_q(1, 1)
        emit_kb_bounce(1, 0, N, nc.gpsimd)
        emit_qb_bounce(1, 0, I, nc.gpsimd)

        # ---------------- filler generators ----------------
        # dwconv (8 non-center taps) + v = x*(1+w4) + pe8, bounced to vdram,
        # then per-key-tile DMA transposes into vbT.
        ROW_CHUNKS = [(0, 12), (12, 12), (24, 12), (36, 4)]
        TAPS8 = [0, 1, 2, 3, 5, 6, 7, 8]

        dw_last = [None]  # last dwconv MM of the most recent items

        def dwconv_gen():
            done_a = [False, False]  # per-ct: cols 0..960 DMA'd
            done_b = [False, False]
            emitted_t1 = False
            emitted_t2 = False
            order = [(0, 0), (0, 1), (1, 0), (1, 1), (0, 2), (0, 3), (1, 2), (1, 3)]
            for ct, chi in order:
                r0, nr = ROW_CHUNKS[chi]
                ps = pp_util.tile([128, 512], dt.float32, tag="util", name="util")
                for ti, tap in enumerate(TAPS8):
                    dh, dw = tap // 3, tap % 3
                    src = xpad[ct][:].rearrange("p (h w) -> p h w", h=PW)[
                        :, r0 + dh : r0 + dh + nr, dw : dw + 40
                    ]
                    dw_last[0] = nc.tensor.matmul(
                        ps[:, : nr * 40],
                        wpe[:, 8 * ct + ti, :],
                        src,
                        start=(ti == 0),
                        stop=(ti == 7),
                    )
                    yield
                nc.vector.scalar_tensor_tensor(
                    vb[ct][:, 40 * r0 : 40 * (r0 + nr)],
                    xb[ct][:, 40 * r0 : 40 * (r0 + nr)],
                    w4p1[ct],
                    ps[:, : nr * 40],
                    op0=ALU.mult,
                    op1=ALU.add,
                )
                if chi == 1:
                    # cols 0..960 of this ct complete -> bounce to vdram
                    nc.sync.dma_start(
                        vdram[:]
                        .rearrange("(h e) w -> h e w", e=VSTRIDE)[
                            4 * ct : 4 * ct + 4, 0:32, 0:960
                        ],
                        vb[ct][:, 0:960],
                    )
                    done_a[ct] = True
                if chi == 3:
                    nc.sync.dma_start(
                        vdram[:]
                        .rearrange("(h e) w -> h e w", e=VSTRIDE)[
                            4 * ct : 4 * ct + 4, 0:32, 960:NPAD
                        ],
                        vb[ct][:, 960:NPAD],
                    )
                    done_b[ct] = True
                if all(done_a) and not emitted_t1:
                    for jt in range(7):
                        nc.sync.dma_start_transpose(
                            vbT[jt][:], vdram[:, 128 * jt : 128 * (jt + 1)]
                        )
                    emitted_t1 = True
                if all(done_b) and not emitted_t2:
                    for jt in range(7, JT):
                        nc.sync.dma_start_transpose(
                            vbT[jt][:], vdram[:, 128 * jt : 128 * (jt + 1)]
                        )
                    emitted_t2 = True
                yield

        # ---------------- attention pipeline ----------------
        proj_ctx = ExitStack()
        pp_proj = None

        def emit_av_pair(p_icx, p_g, p_exs, avts, jt, pr2):
            ic_off, ic = IC[p_icx]
            js = 128 if jt < 12 else 64
            exb = p_exs[2 * jt + pr2][:].bitcast(dt.bfloat16)
            mms = []
            for k2 in range(2):
                hg = 4 * p_g + 2 * pr2 + k2
                mms.append(nc.tensor.matmul(
                    avts[pr2][64 * k2 : 64 * k2 + 33, 0:ic],
                    vbT[jt][0:js, VSTRIDE * hg : VSTRIDE * hg + 33],
                    exb[0:js, k2, 0:ic],
                    start=(jt == 0),
                    stop=(jt == 12),
                    tile_position=(0, 64 * k2),
                    skip_group_check=True,
                ))
            return mms

        def emit_norm_phase1(p_icx, p_g, avts):
            # denominator reciprocal (DVE) -> K=64 ones-matmul broadcast across
            # 32 partitions (PE, borrowed ring tile) -> psum->sbuf copy (ScalarE)
            ic_off, ic = IC[p_icx]
            rb = pp_ring.tile([128, 2, 512], dt.float32, tag="qk", name="qk")
            for t in range(2):
                rstk = work.tile([128, 512], dt.float32, tag="rstk", name="rstk")
                nc.vector.reciprocal_approx_fast(rstk[0:128, 0:ic], avts[t][0:128, 0:ic])
                # bf16 copy so the broadcast matmul avoids slow fp32 PE mode
                rstk16 = work.tile([128, 512], dt.bfloat16, tag="rstk16", name="rstk16")
                nc.vector.tensor_copy(rstk16[0:128, 0:ic], rstk[0:128, 0:ic])
                for sub, base in enumerate((0, 64)):
                    nc.tensor.matmul(
                        rb[64 * sub : 64 * sub + 32, t, 0:ic],
                        bcones[base : base + 64, 0:32],
                        rstk16[base : base + 64, 0:ic],
                        start=True,
                        stop=True,
                        tile_position=(base, 64 * sub),
                        skip_group_check=True,
                    )
            bc = work.tile([96, 2, 512], dt.float32, tag="bc", name="bc")
            # DVE copy: the norm-window exps are routed to ScalarE, so the DVE
            # queue runs this sooner and releases the borrowed ring tile fast
            nc.vector.tensor_copy(bc[0:96, :, 0:ic], rb[0:96, :, 0:ic])
            return bc

        def emit_norm_phase2(p_icx, p_g, avts, bc):
            ic_off, ic = IC[p_icx]
            oTs = oT_all[p_icx]
            for t in range(2):
                p = 2 * p_g + t
                for sub in range(2):
                    nc.vector.tensor_mul(
                        oTs[p][64 * sub : 64 * sub + 32, 0:ic],
                        avts[t][64 * sub : 64 * sub + 32, 0:ic],
                        bc[64 * sub : 64 * sub + 32, t, 0:ic],
                    )

        from concourse.tile_rust import add_dep_helper

        def pin_after(ins_list, anchor):
            if anchor is not None:
                for mm in ins_list:
                    add_dep_helper(mm.ins, anchor.ins, sync=False,
                                   reason="slot ordering")

        def emit_proj(icx, pin=True):
            # borrows a ring buffer for the accumulation (runs at a group
            # boundary; ring-3 cushion absorbs the brief QK stall)
            ic_off, ic = IC[icx]
            oTs = oT_all[icx]
            for ot in range(2):
                ps = pp_ring.tile([128, 2, 512], dt.float32, tag="qk", name="qk")[
                    :, 0, :
                ]
                for p in range(4):
                    nc.tensor.matmul(
                        ps[:, 0:ic],
                        wpr[p][:, 128 * ot : 128 * (ot + 1)],
                        oTs[p][:, 0:ic],
                        start=(p == 0),
                        stop=(p == 3),
                    )
                ob = work.tile([128, 512], dt.float32, tag="ob", name="ob")
                silu_ins = nc.scalar.activation(
                    ob[:, 0:ic], ps[:, 0:ic], AF.Silu, bias=shpj[ot], scale=1.0
                )
                if pin and last_exp[0] is not None:
                    # keep proj SiLUs after the final exp so the scheduler never
                    # interleaves them into the exp stream (act-table thrash)
                    add_dep_helper(silu_ins.ins, last_exp[0].ins, sync=False,
                                   reason="proj silu after exp stream")
                nc.sync.dma_start(
                    out_d.ap()[128 * ot : 128 * (ot + 1), ic_off : ic_off + ic],
                    ob[:, 0:ic],
                )

        dw_it = dwconv_gen()

        def pop(it, n):
            if it is None:
                return None
            for _ in range(n):
                try:
                    next(it)
                except StopIteration:
                    return None
            return it

        pending = None  # (icx, g, exs)
        avts = None
        norm_pending = None
        last_exp = [None]
        copy_pin = [None]
        for gi, (icx, g) in enumerate(GROUPS):
            ic_off, ic = IC[icx]
            exs = []
            if gi == 1:
                util_ctx.close()
                pp_av = av_ctx.enter_context(
                    tc.tile_pool(name="pp_av", bufs=1, space="PSUM")
                )
                avts = [
                    pp_av.tile([128, 512], dt.float32, tag=f"av{t}", name=f"av{t}",
                               bufs=1)
                    for t in range(2)
                ]
                # rows 33:64 / 97:128 are never matmul-written; init for the
                # normalize reads
                for t in range(2):
                    nc.vector.memset(avts[t][:], 1.0)

            # AV pair schedule for the pending group: front-loaded (two pairs
            # per early slot) so the last pair + normalize land well before the
            # group ends and the avts psum is free for the next group
            av_sched = {}
            if pending is not None:
                pairs = [(j, pr) for j in range(JT) for pr in range(2)]
                start_h = 6 if gi == 1 else 2
                end_h = 22 if gi == 1 else (15 if gi == 3 else 20)
                slots = list(range(start_h, end_h))
                extra = len(pairs) - len(slots)
                pi = 0
                for si, hh in enumerate(slots):
                    take = 2 if si < extra else 1
                    av_sched[hh] = pairs[pi : pi + take]
                    pi += take

            slot_anchor = {}  # h -> last filler instruction of that slot
            own_q = [(j, pr) for j in range(JT) for pr in range(2)] if gi == 3 else []
            prev_norm_done = False
            for h in range(26):
                jt, pr = divmod(h, 2)
                js = 128 if jt < 12 else 64
                rb = pp_ring.tile([128, 2, 512], dt.float32, tag="qk", name="qk")
                qk_mms = []
                for k2 in range(2):
                    qk_mms.append(nc.tensor.matmul(
                        rb[0:js, k2, 0:ic],
                        kbp[g][64 * k2 : 64 * k2 + 64, pr, 128 * jt : 128 * jt + js],
                        qbp[g][64 * k2 : 64 * k2 + 64, pr, ic_off : ic_off + ic],
                        start=True,
                        stop=True,
                        tile_position=(64 * k2, 0),
                    ))
                # keep the PE stream alternating: this half's QK runs after
                # slot h-2's fillers
                pin_after(qk_mms, slot_anchor.get(h - 2))
                ex = expool.tile([128, 2, 512], dt.bfloat16, tag="ex", name="ex")
                if _dve_half(icx, g, h):
                    nc.vector.tensor_scalar(
                        ex[:].bitcast(dt.int16)[0:js, :, 0:ic],
                        rb[0:js, :, 0:ic],
                        EXPA,
                        EXPB,
                        op0=ALU.mult,
                        op1=ALU.add,
                    )
                else:
                    last_exp[0] = nc.scalar.activation(
                        ex[0:js, :, 0:ic], rb[0:js, :, 0:ic], AF.Exp, scale=SCALE
                    )
                    if conv_silus:
                        # force every conv SiLU before the first exp so the
                        # scheduler never thrashes the activation table set
                        for si in conv_silus:
                            add_dep_helper(last_exp[0].ins, si.ins, sync=False,
                                           reason="silu before exp stream")
                        conv_silus.clear()
                exs.append(ex)

                # PE slack fillers: all of dwconv lands inside group 0 (popped
                # fast so the v^T bounce + transposes finish well before the
                # front-loaded AV of group 1 needs them; the first slots are
                # kept dwconv-free so the exp stream ramps immediately)
                if gi == 0 and dw_it is not None and h >= 3:
                    dw_it = pop(dw_it, 8)
                if norm_pending is not None and (h == 1 or h >= norm_pending[4]):
                    emit_norm_phase2(*norm_pending[:4])
                    norm_pending = None
                    prev_norm_done = True
                for j, pr in av_sched.get(h, ()):
                    av_mms = emit_av_pair(
                        pending[0], pending[1], pending[2], avts, j, pr
                    )
                    pin_after(av_mms, qk_mms[1])
                    slot_anchor[h] = av_mms[-1]
                    if (j, pr) == (JT - 1, 1):
                        # AV block done: kick off the denominator reciprocal +
                        # broadcast right away; phase2 lands ~3 slots later,
                        # freeing avts before the next group needs it
                        bc = emit_norm_phase1(pending[0], pending[1], avts)
                        norm_pending = (pending[0], pending[1], avts, bc, h + 3)
                if gi == 3 and prev_norm_done and own_q:
                    # previous group normalized: start the final group's own AV
                    # in its remaining slots instead of a long serial drain
                    took = 0
                    while own_q and took < 3 and own_q[0][0] * 2 + own_q[0][1] <= h - 2:
                        j2, pr2 = own_q.pop(0)
                        av_mms = emit_av_pair(icx, g, exs, avts, j2, pr2)
                        pin_after(av_mms, qk_mms[1])
                        slot_anchor[h] = av_mms[-1]
                        took += 1

            pending = (icx, g, exs)

        # drain fillers (shouldn't be any left, but be safe)
        while dw_it is not None:
            dw_it = pop(dw_it, 8)
        # final group's AV + normalize + both projs
        if norm_pending is not None:
            emit_norm_phase2(*norm_pending[:4])
            norm_pending = None
        p_icx, p_g, p_exs = pending
        # prefetch the SiLU act-table set while the norm chain runs: a 1-elem
        # dummy silu right after the last exp hides the ~1.3us table load
        dummy = work.tile([1, 1], dt.float32, tag="dummy", name="dummy")
        dummy_silu = nc.scalar.activation(
            dummy[0:1, 0:1], shpack[0:1, 0:1], AF.Silu, scale=1.0
        )
        if last_exp[0] is not None:
            add_dep_helper(dummy_silu.ins, last_exp[0].ins, sync=False,
                           reason="table prefetch after exp stream")
        for j, pr in own_q:
            emit_av_pair(p_icx, p_g, p_exs, avts, j, pr)
        bc = emit_norm_phase1(p_icx, p_g, avts)
        emit_proj(0)
        emit_norm_phase2(p_icx, p_g, avts, bc)
        emit_proj(1)
        av_ctx.close()

    nc.compile()
    return nc


def _get_nc():
    global _NC_CACHE
    if _NC_CACHE is None:
        _NC_CACHE = _build_nc()
    return _NC_CACHE


def _prep_weights(inputs):
    f32 = np.float32
    qkv_w = np.asarray(inputs["qkv_w"], f32)
    qinv = np.asarray(inputs["qkv_gamma"], f32) / np.sqrt(
        np.asarray(inputs["qkv_var"], f32) + EPS
    )
    wqkvT = np.ascontiguousarray((qkv_w * qinv[:, None]).T.astype(BF16))
    shqkv = (
        np.asarray(inputs["qkv_beta"], f32) - np.asarray(inputs["qkv_mean"], f32) * qinv
    ).astype(f32)[:, None]

    pe_w = np.asarray(inputs["pe_w"], f32)  # [256, 1, 3, 3]
    peinv = np.asarray(inputs["pe_gamma"], f32) / np.sqrt(
        np.asarray(inputs["pe_var"], f32) + EPS
    )
    wpe_f = (pe_w[:, 0] * peinv[:, None, None]).reshape(C, 9)
    shpe = (
        np.asarray(inputs["pe_beta"], f32) - np.asarray(inputs["pe_mean"], f32) * peinv
    ).astype(f32)
    taps8 = [0, 1, 2, 3, 5, 6, 7, 8]
    wpe8 = np.zeros((16, 128, 128), BF16)
    for ct in range(2):
        for ti, tap in enumerate(taps8):
            np.fill_diagonal(
                wpe8[8 * ct + ti], wpe_f[128 * ct : 128 * (ct + 1), tap].astype(BF16)
            )
    # partition-major so the device DMA is contiguous (no strided rearrange)
    wpe8 = np.ascontiguousarray(wpe8.transpose(1, 0, 2))
    w4p1 = (1.0 + wpe_f[:, 4]).astype(f32)[:, None]

    proj_w = np.asarray(inputs["proj_w"], f32)
    pinv = np.asarray(inputs["proj_gamma"], f32) / np.sqrt(
        np.asarray(inputs["proj_var"], f32) + EPS
    )
    wfold = proj_w * pinv[:, None]          # [out, in]
    wprojT = wfold.T.astype(f32)            # [in, out]
    wprojs = np.zeros((4, 128, C), BF16)
    for p in range(4):
        wprojs[p, 0:32] = wprojT[64 * p : 64 * p + 32].astype(BF16)
        wprojs[p, 64:96] = wprojT[64 * p + 32 : 64 * p + 64].astype(BF16)
    # fold v's BN shift through proj: softmax weights sum to one, so the
    # constant shpe offset on v becomes wfold @ shpe added to the proj bias.
    shproj = (
        np.asarray(inputs["proj_beta"], f32)
        - np.asarray(inputs["proj_mean"], f32) * pinv
        + wfold @ shpe
    ).astype(f32)[:, None]

    # packed [128, 8]: cols 0-3 shqkv chunks, 4-5 shproj chunks, 6-7 w4p1
    shpack = np.zeros((128, 8), f32)
    for ot in range(4):
        shpack[:, ot] = shqkv[128 * ot : 128 * (ot + 1), 0]
    for ot in range(2):
        shpack[:, 4 + ot] = shproj[128 * ot : 128 * (ot + 1), 0]
    for ct in range(2):
        shpack[:, 6 + ct] = w4p1[128 * ct : 128 * (ct + 1), 0]

    return dict(wqkvT=wqkvT, wprojs=wprojs, wpe8=wpe8, shpack=shpack)


def build_in_maps(inputs):
    w = _prep_weights(inputs)
    x = np.asarray(inputs["x"], np.float32)  # [4, 256, 40, 40]
    in_maps = []
    for core in range(8):
        b, hf = divmod(core, 2)
        xr = np.ascontiguousarray(x[b].reshape(C, N))
        xb16 = xr.astype(BF16)
        xp = np.zeros((C, PW, PW), BF16)
        xp[:, 1:41, 1:41] = xb16.reshape(C, 40, 40)
        m = {
            "x": np.ascontiguousarray(xb16),
            "xq": np.ascontiguousarray(xb16[:, I * hf : I * (hf + 1)]),
            "xpad": np.ascontiguousarray(xp.reshape(C, PADN)),
        }
        m.update(w)
        in_maps.append(m)
    return in_maps


def assemble(results):
    out = np.empty((4, C, 40, 40), np.float32)
    for core in range(8):
        b, hf = divmod(core, 2)
        o = np.asarray(results[core]["out"], np.float32)
        out[b].reshape(C, N)[:, I * hf : I * (hf + 1)] = o
    return out


def _install_ntff_hook():
    """Provide antenv.axon_hooks (missing in this image) so trace=True works."""
    import types

    try:
        import antenv.axon_hooks  # noqa: F401
        return
    except ImportError:
        pass
    import antenv

    mod = types.ModuleType("antenv.axon_hooks")
    state = {"hook": None}
    mod.set_axon_ntff_profile_hook = lambda h: state.__setitem__("hook", h)
    mod.get_axon_ntff_profile_hook = lambda: state["hook"]
    sys.modules["antenv.axon_hooks"] = mod
    antenv.axon_hooks = mod

    so_path = "/opt/axon/libaxon_pjrt.so"
    if os.path.exists(so_path):
        boot_dir = "/root/.axon_site/trn_agent_boot"
        if boot_dir not in sys.path and os.path.isdir(boot_dir):
            sys.path.append(boot_dir)
        try:
            from trn_boot import _ntff_profile_via_ctypes

            mod.set_axon_ntff_profile_hook(_ntff_profile_via_ctypes(so_path))
        except Exception as e:  # pragma: no cover
            print(f"ntff hook install failed: {e}", file=sys.stderr)


def kernel(**inputs):
    global LAST_EXEC_NS
    _install_ntff_hook()
    from concourse.bass_utils import run_bass_kernel_spmd

    nc = _get_nc()
    in_maps = build_in_maps(inputs)
    trace = bool(int(os.environ.get("KERNEL_TRACE", "0")))
    res = run_bass_kernel_spmd(nc, in_maps, core_ids=list(range(8)), trace=trace)
    LAST_EXEC_NS = res.exec_time_ns
    return assemble(res.results)

